# revision 2
# baseline (speedup 1.0000x reference)
"""DeBERTa-RoPE self-attention on 8 Trainium2 cores — v4 (bf16 streaming).

Sharding: data-parallel over batch (4) x tensor-parallel over heads (2 groups
of 8); core = 2*b + g. Host sums the two row-parallel out-proj partials.

All matmul paths run in bf16 (f32 PSUM accumulation); fp8 was measured to
break the 2e-2 gate. The schedule keeps the PE dense end-to-end:

  seg1: qk-proj pairs 0,1 (psum = scores-pool tiles viewed flat)
  w0:   scores+exp pair 0   | qk-proj pairs 2,3 (single rotating bank-pair)
  w1:   scores+exp pair 1   | v-projection (2-bank rolling)
  w2:   scores+exp pair 2   | ctx pair 0 (+softmax-normalize)
  w3:   scores+exp pair 3   | ctx pairs 1,2
  tail: ctx pair 3, out-projection (psum = scores-pool tiles again)

The attention mask folds into the exp bias (masked keys exp to ~0); the
denominator rides as a 65th stationary column of v. exps live in SBUF bf16,
two pair-sized buffers rotating.
"""

import numpy as np
import ml_dtypes

import concourse.bass as bass
import concourse.mybir as mybir
import concourse.tile as tile
from concourse.bass_utils import run_bass_kernel_spmd

H = 16
D = 64
HID = H * D
B = 4
S = 1024
THETA = 10000.0
NCORES = 8
HPC = H // 2          # heads per core
KT = HID // 128       # 8 k-tiles
ST = S // 128         # 8 seq tiles

F32 = mybir.dt.float32
F32R = mybir.dt.float32r
BF16 = mybir.dt.bfloat16
AF = mybir.ActivationFunctionType
ALU = mybir.AluOpType

NEGB = -30.0          # bias for masked keys: exp(s*0.125 - 30) ~ 0


def _r(ap):
    return ap.bitcast(F32R)


def build_program():
    nc = bass.Bass()
    xT = nc.declare_dram_parameter("xT", [HID, S], BF16, isOutput=False)
    wqk = nc.declare_dram_parameter("wqk", [4, HID, 256], BF16, isOutput=False)
    wv = nc.declare_dram_parameter("wv", [HID, 512], BF16, isOutput=False)
    bqk = nc.declare_dram_parameter("bqk", [128, 8], F32, isOutput=False)
    bqksh = nc.declare_dram_parameter("bqksh", [128, 8], F32, isOutput=False)
    cosT = nc.declare_dram_parameter("cosT", [128, S], BF16, isOutput=False)
    sinT = nc.declare_dram_parameter("sinT", [128, S], BF16, isOutput=False)
    mbias = nc.declare_dram_parameter("mbias", [128, ST], F32, isOutput=False)
    wout = nc.declare_dram_parameter("wout", [512, HID], BF16, isOutput=False)
    permT = nc.declare_dram_parameter("permT", [128, 128], BF16, isOutput=False)
    yT = nc.declare_dram_parameter("yT", [HID, S], BF16, isOutput=True)

    with tile.TileContext(nc) as tc, nc.allow_low_precision(
            reason="bf16 recip/cos tables; rel-err budget 2e-2"):
        with (
            tc.tile_pool(name="const", bufs=1) as cpool,
            tc.tile_pool(name="persist", bufs=1) as persist,
        ):
            cos_sb = cpool.tile([128, S], BF16)
            sin_sb = cpool.tile([128, S], BF16)
            mbias_sb = cpool.tile([128, ST], F32)
            bqk_sb = cpool.tile([128, 8], F32)
            bqksh_sb = cpool.tile([128, 8], F32)
            permT_sb = cpool.tile([128, 128], BF16)

            rope = [persist.tile([128, S], BF16, name=f"rope{i}")
                    for i in range(8)]
            # v + denominator-ones column: [t, tt, head, 65]
            vm = persist.tile([128, ST, HPC, 65], BF16)
            ctxn = [persist.tile([128, S], BF16, name=f"ctxn{i}")
                    for i in range(4)]
            xT_sb = persist.tile([128, KT, S], BF16)
            wqk_sb = persist.tile([128, KT, 4, 256], BF16)
            wv_sb = persist.tile([128, KT, 512], BF16)
            wout_sb = persist.tile([128, 4, HID], BF16)

            # ---- input DMAs ----
            nc.scalar.dma_start(cos_sb[:], cosT[:])
            nc.scalar.dma_start(sin_sb[:], sinT[:])
            nc.scalar.dma_start(mbias_sb[:], mbias[:])
            nc.scalar.dma_start(bqk_sb[:], bqk[:])
            nc.scalar.dma_start(bqksh_sb[:], bqksh[:])
            nc.scalar.dma_start(permT_sb[:], permT[:])
            nc.sync.dma_start(
                wqk_sb[:, :, 0, :],
                wqk[0].rearrange("(k p) n -> p k n", p=128))
            for c in range(4):
                nc.sync.dma_start(
                    xT_sb[:, 2 * c:2 * c + 2, :],
                    xT[c * 256:(c + 1) * 256, :].rearrange(
                        "(k p) s -> p k s", p=128))
            nc.sync.dma_start(
                wqk_sb[:, :, 1, :],
                wqk[1].rearrange("(k p) n -> p k n", p=128))
            nc.scalar.dma_start(
                wv_sb[:], wv[:].rearrange("(k p) n -> p k n", p=128))
            for q_ in (2, 3):
                nc.sync.dma_start(
                    wqk_sb[:, :, q_, :],
                    wqk[q_].rearrange("(k p) n -> p k n", p=128))
            nc.sync.dma_start(
                wout_sb[:], wout[:].rearrange("(k p) n -> p k n", p=128))

            # denominator ones-column
            nc.gpsimd.memset(vm[:, :, :, 64:65], 1.0)

            with (
                tc.tile_pool(name="qksb", bufs=1) as qkp,
                tc.tile_pool(name="ropetmp", bufs=1) as rt,
                tc.tile_pool(name="expool", bufs=1) as exp_pool,
                tc.tile_pool(name="tailp", bufs=2) as tp,
                tc.tile_pool(name="small", bufs=1) as small,
                tc.tile_pool(name="rbp", bufs=1) as rbp,
                tc.tile_pool(name="drbounce", bufs=2, space="DRAM") as drb,
                tc.tile_pool(name="psS", bufs=1, space="PSUM") as psS,
            ):
                def sc_tile(name_, tag):
                    return psS.tile([128, 2, 512], F32, tag=tag, name=name_)

                def qk_unit_mms_half(m, ps_qk, half):
                    for kt in range(4 * half, 4 * half + 4):
                        for ch in range(2):
                            nc.tensor.matmul(
                                ps_qk[:, ch * 512:(ch + 1) * 512],
                                wqk_sb[:, kt, m % 4, (m // 4) * 128:
                                       (m // 4) * 128 + 128],
                                xT_sb[:, kt, ch * 512:(ch + 1) * 512],
                                start=(kt == 0), stop=(kt == KT - 1),
                            )

                def qk_unit_mms(m, ps_qk):
                    for kt in range(KT):
                        for ch in range(2):
                            nc.tensor.matmul(
                                ps_qk[:, ch * 512:(ch + 1) * 512],
                                wqk_sb[:, kt, m % 4, (m // 4) * 128:
                                       (m // 4) * 128 + 128],
                                xT_sb[:, kt, ch * 512:(ch + 1) * 512],
                                start=(kt == 0), stop=(kt == KT - 1),
                            )

                def qk_unit_tail(m, ps_qk, psSh):
                    qk_sb = qkp.tile([128, S], BF16, tag="qksb",
                                     name=f"qksb{m}")
                    nc.vector.tensor_copy(qk_sb[:], ps_qk[:])
                    ps_sh = psSh.tile([128, S], F32, tag="sh",
                                      name=f"pssh{m}")
                    for ch in range(2):
                        nc.tensor.matmul(
                            ps_sh[:, ch * 512:(ch + 1) * 512],
                            permT_sb[:],
                            qk_sb[:, ch * 512:(ch + 1) * 512],
                            start=True, stop=True,
                        )
                    for ch in range(2):
                        sl = slice(ch * 512, (ch + 1) * 512)
                        t1 = rt.tile([128, 512], BF16, tag=f"t1{ch}")
                        nc.vector.scalar_tensor_tensor(
                            t1[:], ps_qk[:, sl], bqk_sb[:, m:m + 1],
                            cos_sb[:, sl], op0=ALU.add, op1=ALU.mult)
                        s2 = rt.tile([128, 512], BF16, tag=f"s2{ch}")
                        nc.vector.scalar_tensor_tensor(
                            s2[:], ps_sh[:, sl], bqksh_sb[:, m:m + 1],
                            sin_sb[:, sl], op0=ALU.add, op1=ALU.mult)
                        nc.vector.tensor_add(
                            rope[m][:, sl], t1[:], s2[:])

                def scores_tt(p, tt, ps_sc, ex):
                    qp = rope[p][:]
                    kp = rope[p + 4][:]
                    for ch in range(2):
                        for hh in range(2):
                            base = hh * 64
                            nc.tensor.matmul(
                                ps_sc[ch][:, hh, :],
                                kp[base:base + 64, tt * 128:(tt + 1) * 128],
                                qp[base:base + 64, ch * 512:(ch + 1) * 512],
                                start=True, stop=True,
                                tile_position=(base, 0),
                            )
                        nc.scalar.activation(
                            ex[:, tt, :, ch, :], ps_sc[ch][:],
                            AF.Exp, bias=mbias_sb[:, tt:tt + 1], scale=0.125)

                def new_ex(p):
                    # exps for one pair: [t, tt, hh, ch, s]; 2 rotating bufs
                    return exp_pool.tile([128, ST, 2, 2, 512], BF16,
                                         tag=f"ex{p % 3}", name=f"ex{p}")

                def ctx_pair(p, ex, psC):
                    ps_cs = [psC.tile([65, S], F32, tag=f"ctx{hh}",
                                      name=f"ctx{p}_{hh}")
                             for hh in range(2)]
                    for tt in range(ST):
                        for hh in range(2):
                            for ch in range(2):
                                nc.tensor.matmul(
                                    ps_cs[hh][:, ch * 512:(ch + 1) * 512],
                                    vm[:, tt, 2 * p + hh, :],
                                    ex[:, tt, hh, ch, :],
                                    start=(tt == 0), stop=(tt == ST - 1),
                                )
                    # recip the denominators, evacuate unnormalized ctx to
                    # SBUF (frees the psum tag fast), then normalize SBUF-side
                    rc0 = small.tile([1, S], BF16, tag="recip0")
                    rc1 = small.tile([1, S], BF16, tag="recip1")
                    nc.vector.reciprocal(rc0[:], ps_cs[0][64:65, :])
                    nc.vector.reciprocal(rc1[:], ps_cs[1][64:65, :])
                    cu0 = small.tile([64, S], BF16, tag="cun0")
                    cu1 = small.tile([64, S], BF16, tag="cun1")
                    nc.scalar.copy(cu0[:], ps_cs[0][0:64, :])
                    nc.vector.tensor_copy(cu1[:], ps_cs[1][0:64, :])
                    bounce = drb.tile([2, S], BF16)
                    rb = rbp.tile([64, 2, S], BF16, tag="rb")
                    nc.sync.dma_start(bounce[0:1, :], rc0[:])
                    nc.sync.dma_start(
                        rb[:, 0, :], bounce[0:1, :].broadcast_to([64, S]))
                    nc.gpsimd.dma_start(bounce[1:2, :], rc1[:])
                    nc.gpsimd.dma_start(
                        rb[:, 1, :], bounce[1:2, :].broadcast_to([64, S]))
                    nc.vector.tensor_mul(
                        ctxn[p][0:64, :], cu0[:], rb[:, 0, :])
                    nc.vector.tensor_mul(
                        ctxn[p][64:128, :], cu1[:], rb[:, 1, :])

                # ================= seg 1: qk pairs 0,1 =================
                with (
                    tc.tile_pool(name="psA2", bufs=1, space="PSUM") as psA2,
                    tc.tile_pool(name="psSh", bufs=1, space="PSUM") as psSh,
                ):
                    seg1_ps = {}
                    for i, m in enumerate((4, 0, 5, 1)):
                        # qk psums borrow the scores-pool tags (viewed flat)
                        if i % 2 == 0:
                            ps_q = sc_tile(f"qkps{m}", f"sc{i // 2}")
                        else:
                            ps_q = psA2.tile([128, 2, 512], F32, tag="qk",
                                             name=f"qkps{m}")
                        fl = ps_q[:].rearrange("p a b -> p (a b)")
                        qk_unit_mms(m, fl)
                        qk_unit_tail(m, fl, psSh)

                    # ============ w0: stream pair 0 + qk pairs 2,3 =========
                    ps_sc = (sc_tile("sc0_0", "sc0"), sc_tile("sc1_0", "sc1"))
                    ex0 = new_ex(0)
                    for tt in range(ST):
                        scores_tt(0, tt, ps_sc, ex0)
                        m = (6, 2, 7, 3)[tt // 2]
                        if tt % 2 == 0:
                            psq = psA2.tile([128, 2, 512], F32, tag="qk",
                                            name=f"qkps{m}")
                        qk_unit_mms_half(m, psq[:].rearrange(
                            "p a b -> p (a b)"), tt % 2)
                        if tt % 2 == 1:
                            qk_unit_tail(m, psq[:].rearrange(
                                "p a b -> p (a b)"), psSh)

                # ============ w1: stream pair 1 + v projection =============
                with tc.tile_pool(name="psV", bufs=1, space="PSUM") as psV:
                    ps_sc = (sc_tile("sc0_1", "sc0"), sc_tile("sc1_1", "sc1"))
                    ex1 = new_ex(1)
                    for tt in range(ST):
                        scores_tt(1, tt, ps_sc, ex1)
                        vps = psV.tile([128, 512], F32, tag=f"v{tt % 2}",
                                       name=f"vps{tt}")
                        for kt in range(KT):
                            nc.tensor.matmul(
                                vps[:],
                                xT_sb[:, kt, tt * 128:(tt + 1) * 128],
                                wv_sb[:, kt, :],
                                start=(kt == 0), stop=(kt == KT - 1),
                            )
                        nc.scalar.copy(
                            vm[:, tt, :, 0:64],
                            vps[:].rearrange("p (h d) -> p h d", d=64))

                # ===== w2, w3: stream pairs 2,3 + ctx pairs 0,1,2 ==========
                with tc.tile_pool(name="psC", bufs=1, space="PSUM") as psC:
                    ps_sc = (sc_tile("sc0_2", "sc0"), sc_tile("sc1_2", "sc1"))
                    ex2 = new_ex(2)
                    for tt in range(ST):
                        scores_tt(2, tt, ps_sc, ex2)
                    ctx_pair(0, ex0, psC)

                    ps_sc = (sc_tile("sc0_3", "sc0"), sc_tile("sc1_3", "sc1"))
                    ex3 = new_ex(3)
                    for tt in range(ST):
                        scores_tt(3, tt, ps_sc, ex3)
                    ctx_pair(1, ex1, psC)
                    ctx_pair(2, ex2, psC)

                    # ================= tail: ctx 3 + out-projection ========
                    ctx_pair(3, ex3, psC)

                    for grp in range(4):
                        mg = range(grp * 2, grp * 2 + 2)
                        psy = {m: sc_tile(f"psy{m}", f"sc{i}")[:].rearrange(
                            "p a b -> p (a b)") for i, m in enumerate(mg)}
                        for kt in range(4):
                            for m in mg:
                                for ch in range(2):
                                    nc.tensor.matmul(
                                        psy[m][:, ch * 512:(ch + 1) * 512],
                                        wout_sb[:, kt,
                                                m * 128:(m + 1) * 128],
                                        ctxn[kt][:,
                                                 ch * 512:(ch + 1) * 512],
                                        start=(kt == 0), stop=(kt == 3),
                                    )
                        for m in mg:
                            y_sb = tp.tile([128, S], BF16, tag="ysb",
                                           name=f"ysb{m}")
                            if m % 2 == 0:
                                nc.scalar.copy(y_sb[:], psy[m])
                            else:
                                nc.vector.tensor_copy(y_sb[:], psy[m])
                            nc.sync.dma_start(
                                yT[m * 128:(m + 1) * 128, :], y_sb[:])

    return nc


def _split_waits(nc, max_waits=1):
    """This walrus build rejects >1 sync-wait command per instruction; hoist
    extra waits onto preceding NoOps on the same engine/queue."""
    for bb in nc.main_func.blocks:
        new_insts = []
        for ins in bb.instructions:
            si = getattr(ins, "sync_info", None)
            if si is not None and si.on_wait and len(si.on_wait) > max_waits:
                waits = list(si.on_wait)
                head, rest = waits[:max_waits], waits[max_waits:]
                while rest:
                    chunk, rest = rest[:max_waits], rest[max_waits:]
                    new_insts.append(mybir.InstNoOp(
                        name=f"waitsplit-{nc.next_id()}", ins=[], outs=[],
                        sync_info=mybir.SyncInfo(on_wait=chunk, on_update=[]),
                        engine=ins.engine))
                ins.sync_info = mybir.SyncInfo(
                    on_wait=head, on_update=list(si.on_update or []))
            new_insts.append(ins)
        bb.instructions = new_insts


def make_core_inputs(x, attention_mask, Wqkv, bqkv, Wout):
    """Host-side shard prep: returns list of 8 in_maps (core = 2*b + g)."""
    Wr = np.ascontiguousarray(Wqkv).reshape(HID, 3, H, D)
    br = np.ascontiguousarray(bqkv).reshape(3, H, D)

    inv = 1.0 / (THETA ** (np.arange(0, D, 2, dtype=np.float64) / D))
    pos = np.arange(S, dtype=np.float64)
    freqs = pos[:, None] * inv[None, :]              # [S, 32]
    emb = np.concatenate([freqs, freqs], axis=1)     # [S, 64]
    cosT = np.cos(emb).T.astype(np.float32)          # [64, S]
    sgn = np.concatenate([-np.ones(32), np.ones(32)])[:, None]
    sinTs = (sgn * np.sin(emb).T).astype(np.float32)
    cos2 = np.concatenate([cosT, cosT], 0)           # [128, S]
    sin2 = np.concatenate([sinTs, sinTs], 0)

    pp = np.arange(128)
    shmap = (pp - pp % 64) + (pp % 64 + 32) % 64
    permT = np.zeros((128, 128), dtype=np.float32)
    permT[shmap, pp] = 1.0

    in_maps = []
    for c in range(NCORES):
        b, g = c // 2, c % 2
        # wqk grouped per head pair: [pair, HID, (q 128 | k 128)]
        wqk_ = np.empty((4, HID, 256), dtype=ml_dtypes.bfloat16)
        for p_ in range(4):
            hs = slice(g * HPC + 2 * p_, g * HPC + 2 * p_ + 2)
            wqk_[p_, :, 0:128] = Wr[:, 0, hs, :].reshape(HID, 128)
            wqk_[p_, :, 128:256] = Wr[:, 1, hs, :].reshape(HID, 128)
        hsg = slice(g * HPC, (g + 1) * HPC)
        wv_ = Wr[:, 2, hsg, :].reshape(HID, 512)
        bqk_ = np.concatenate(
            [br[0, hsg].reshape(512), br[1, hsg].reshape(512)]
        ).reshape(8, 128).T
        bqksh_ = bqk_[shmap]
        mb = (NEGB * (1.0 - attention_mask[b].astype(np.float32))
              ).reshape(ST, 128).T
        in_maps.append({
            "xT": np.ascontiguousarray(x[b].T.astype(ml_dtypes.bfloat16)),
            "wqk": np.ascontiguousarray(wqk_),
            "wv": np.ascontiguousarray(wv_.astype(ml_dtypes.bfloat16)),
            "bqk": np.ascontiguousarray(bqk_.astype(np.float32)),
            "bqksh": np.ascontiguousarray(bqksh_.astype(np.float32)),
            "permT": permT.astype(ml_dtypes.bfloat16),
            "cosT": cos2.astype(ml_dtypes.bfloat16),
            "sinT": sin2.astype(ml_dtypes.bfloat16),
            "mbias": np.ascontiguousarray(mb.astype(np.float32)),
            "wout": np.ascontiguousarray(
                Wout[g * 512:(g + 1) * 512, :].astype(ml_dtypes.bfloat16)),
        })
    return in_maps


_PROGRAM = None


def kernel(x, attention_mask, Wqkv, bqkv, Wout, bout, _trace=False):
    global _PROGRAM
    x = np.asarray(x)
    attention_mask = np.asarray(attention_mask)
    Wqkv = np.asarray(Wqkv)
    bqkv = np.asarray(bqkv)
    Wout = np.asarray(Wout)
    bout = np.asarray(bout)

    if _PROGRAM is None:
        _PROGRAM = build_program()
        _split_waits(_PROGRAM)
    nc = _PROGRAM

    in_maps = make_core_inputs(x, attention_mask, Wqkv, bqkv, Wout)
    res = run_bass_kernel_spmd(
        nc, in_maps, core_ids=list(range(NCORES)), trace=_trace)

    y = np.empty((B, S, HID), dtype=np.float32)
    for b in range(B):
        acc = (res.results[2 * b]["yT"].astype(np.float32)
               + res.results[2 * b + 1]["yT"].astype(np.float32))
        y[b] = acc.T
    # exact host-side bias corrections: v-bias shifts context by a constant
    # (attn rows sum to 1), q/k biases were applied on device.
    bv = bqkv[2 * HID:3 * HID].astype(np.float32)
    y += (bv @ Wout + bout).astype(np.float32)[None, None, :]
    if _trace:
        kernel.last_exec_time_ns = res.exec_time_ns
    return y


# revision 3
# speedup vs baseline: 1.0081x; 1.0081x over previous
"""DeBERTa-RoPE self-attention on 8 Trainium2 cores — v4 (bf16 streaming).

Sharding: data-parallel over batch (4) x tensor-parallel over heads (2 groups
of 8); core = 2*b + g. Host sums the two row-parallel out-proj partials.

All matmul paths run in bf16 (f32 PSUM accumulation); fp8 was measured to
break the 2e-2 gate. The schedule keeps the PE dense end-to-end:

  seg1: qk-proj pairs 0,1 (psum = scores-pool tiles viewed flat)
  w0:   scores+exp pair 0   | qk-proj pairs 2,3 (single rotating bank-pair)
  w1:   scores+exp pair 1   | v-projection (2-bank rolling)
  w2:   scores+exp pair 2   | ctx pair 0 (+softmax-normalize)
  w3:   scores+exp pair 3   | ctx pairs 1,2
  tail: ctx pair 3, out-projection (psum = scores-pool tiles again)

The attention mask folds into the exp bias (masked keys exp to ~0); the
denominator rides as a 65th stationary column of v. exps live in SBUF bf16,
two pair-sized buffers rotating.
"""

import numpy as np
import ml_dtypes

import concourse.bass as bass
import concourse.mybir as mybir
import concourse.tile as tile
from concourse.bass_utils import run_bass_kernel_spmd

H = 16
D = 64
HID = H * D
B = 4
S = 1024
THETA = 10000.0
NCORES = 8
HPC = H // 2          # heads per core
KT = HID // 128       # 8 k-tiles
ST = S // 128         # 8 seq tiles

F32 = mybir.dt.float32
F32R = mybir.dt.float32r
BF16 = mybir.dt.bfloat16
AF = mybir.ActivationFunctionType
ALU = mybir.AluOpType

NEGB = -30.0          # bias for masked keys: exp(s*0.125 - 30) ~ 0


def _r(ap):
    return ap.bitcast(F32R)


def build_program():
    nc = bass.Bass()
    xT = nc.declare_dram_parameter("xT", [HID, S], BF16, isOutput=False)
    wqk = nc.declare_dram_parameter("wqk", [4, HID, 256], BF16, isOutput=False)
    wv = nc.declare_dram_parameter("wv", [HID, 512], BF16, isOutput=False)
    bqk = nc.declare_dram_parameter("bqk", [128, 8], F32, isOutput=False)
    bqksh = nc.declare_dram_parameter("bqksh", [128, 8], F32, isOutput=False)
    cosT = nc.declare_dram_parameter("cosT", [128, S], BF16, isOutput=False)
    sinT = nc.declare_dram_parameter("sinT", [128, S], BF16, isOutput=False)
    mbias = nc.declare_dram_parameter("mbias", [128, ST], F32, isOutput=False)
    wout = nc.declare_dram_parameter("wout", [512, HID], BF16, isOutput=False)
    permT = nc.declare_dram_parameter("permT", [128, 128], BF16, isOutput=False)
    yT = nc.declare_dram_parameter("yT", [HID, S], BF16, isOutput=True)

    with tile.TileContext(nc) as tc, nc.allow_low_precision(
            reason="bf16 recip/cos tables; rel-err budget 2e-2"):
        with (
            tc.tile_pool(name="const", bufs=1) as cpool,
            tc.tile_pool(name="persist", bufs=1) as persist,
        ):
            cos_sb = cpool.tile([128, S], BF16)
            sin_sb = cpool.tile([128, S], BF16)
            mbias_sb = cpool.tile([128, ST], F32)
            bqk_sb = cpool.tile([128, 8], F32)
            bqksh_sb = cpool.tile([128, 8], F32)
            permT_sb = cpool.tile([128, 128], BF16)

            rope = [persist.tile([128, S], BF16, name=f"rope{i}")
                    for i in range(8)]
            # v + denominator-ones column: [t, tt, head, 65]
            vm = persist.tile([128, ST, HPC, 65], BF16)
            ctxn = [persist.tile([128, S], BF16, name=f"ctxn{i}")
                    for i in range(4)]
            xT_sb = persist.tile([128, KT, S], BF16)
            wqk_sb = persist.tile([128, KT, 4, 256], BF16)
            wv_sb = persist.tile([128, KT, 512], BF16)
            wout_sb = persist.tile([128, 4, HID], BF16)

            # ---- input DMAs ----
            nc.scalar.dma_start(cos_sb[:], cosT[:])
            nc.scalar.dma_start(sin_sb[:], sinT[:])
            nc.scalar.dma_start(mbias_sb[:], mbias[:])
            nc.scalar.dma_start(bqk_sb[:], bqk[:])
            nc.scalar.dma_start(bqksh_sb[:], bqksh[:])
            nc.scalar.dma_start(permT_sb[:], permT[:])
            nc.sync.dma_start(
                wqk_sb[:, :, 0, :],
                wqk[0].rearrange("(k p) n -> p k n", p=128))
            for c in range(4):
                nc.sync.dma_start(
                    xT_sb[:, 2 * c:2 * c + 2, :],
                    xT[c * 256:(c + 1) * 256, :].rearrange(
                        "(k p) s -> p k s", p=128))
            nc.sync.dma_start(
                wqk_sb[:, :, 1, :],
                wqk[1].rearrange("(k p) n -> p k n", p=128))
            nc.scalar.dma_start(
                wv_sb[:], wv[:].rearrange("(k p) n -> p k n", p=128))
            for q_ in (2, 3):
                nc.sync.dma_start(
                    wqk_sb[:, :, q_, :],
                    wqk[q_].rearrange("(k p) n -> p k n", p=128))
            nc.sync.dma_start(
                wout_sb[:], wout[:].rearrange("(k p) n -> p k n", p=128))

            # denominator ones-column
            nc.gpsimd.memset(vm[:, :, :, 64:65], 1.0)

            with (
                tc.tile_pool(name="qksb", bufs=1) as qkp,
                tc.tile_pool(name="ropetmp", bufs=1) as rt,
                tc.tile_pool(name="expool", bufs=1) as exp_pool,
                tc.tile_pool(name="tailp", bufs=2) as tp,
                tc.tile_pool(name="small", bufs=1) as small,
                tc.tile_pool(name="rbp", bufs=1) as rbp,
                tc.tile_pool(name="drbounce", bufs=2, space="DRAM") as drb,
                tc.tile_pool(name="psS", bufs=1, space="PSUM") as psS,
            ):
                def sc_tile(name_, tag):
                    return psS.tile([128, 2, 512], F32, tag=tag, name=name_)

                def qk_unit_mms_half(m, ps_qk, half):
                    for kt in range(4 * half, 4 * half + 4):
                        for ch in range(2):
                            nc.tensor.matmul(
                                ps_qk[:, ch * 512:(ch + 1) * 512],
                                wqk_sb[:, kt, m % 4, (m // 4) * 128:
                                       (m // 4) * 128 + 128],
                                xT_sb[:, kt, ch * 512:(ch + 1) * 512],
                                start=(kt == 0), stop=(kt == KT - 1),
                            )

                def qk_unit_mms(m, ps_qk):
                    for kt in range(KT):
                        for ch in range(2):
                            nc.tensor.matmul(
                                ps_qk[:, ch * 512:(ch + 1) * 512],
                                wqk_sb[:, kt, m % 4, (m // 4) * 128:
                                       (m // 4) * 128 + 128],
                                xT_sb[:, kt, ch * 512:(ch + 1) * 512],
                                start=(kt == 0), stop=(kt == KT - 1),
                            )

                def qk_unit_tail(m, ps_qk, psSh):
                    qk_sb = qkp.tile([128, S], BF16, tag="qksb",
                                     name=f"qksb{m}")
                    nc.vector.tensor_copy(qk_sb[:], ps_qk[:])
                    ps_sh = psSh.tile([128, S], F32, tag="sh",
                                      name=f"pssh{m}")
                    for ch in range(2):
                        nc.tensor.matmul(
                            ps_sh[:, ch * 512:(ch + 1) * 512],
                            permT_sb[:],
                            qk_sb[:, ch * 512:(ch + 1) * 512],
                            start=True, stop=True,
                        )
                    for ch in range(2):
                        sl = slice(ch * 512, (ch + 1) * 512)
                        t1 = rt.tile([128, 512], BF16, tag=f"t1{ch}")
                        nc.vector.scalar_tensor_tensor(
                            t1[:], ps_qk[:, sl], bqk_sb[:, m:m + 1],
                            cos_sb[:, sl], op0=ALU.add, op1=ALU.mult)
                        s2 = rt.tile([128, 512], BF16, tag=f"s2{ch}")
                        nc.vector.scalar_tensor_tensor(
                            s2[:], ps_sh[:, sl], bqksh_sb[:, m:m + 1],
                            sin_sb[:, sl], op0=ALU.add, op1=ALU.mult)
                        nc.vector.tensor_add(
                            rope[m][:, sl], t1[:], s2[:])

                def scores_tt(p, tt, ps_sc, ex):
                    qp = rope[p][:]
                    kp = rope[p + 4][:]
                    for ch in range(2):
                        for hh in range(2):
                            base = hh * 64
                            nc.tensor.matmul(
                                ps_sc[ch][:, hh, :],
                                kp[base:base + 64, tt * 128:(tt + 1) * 128],
                                qp[base:base + 64, ch * 512:(ch + 1) * 512],
                                start=True, stop=True,
                                tile_position=(base, 0),
                            )
                        nc.scalar.activation(
                            ex[:, tt, :, ch, :], ps_sc[ch][:],
                            AF.Exp, bias=mbias_sb[:, tt:tt + 1], scale=0.125)

                def new_ex(p):
                    # exps for one pair: [t, tt, hh, ch, s]; 2 rotating bufs
                    return exp_pool.tile([128, ST, 2, 2, 512], BF16,
                                         tag=f"ex{p % 3}", name=f"ex{p}")

                def ctx_alloc(p, psC):
                    return [psC.tile([65, S], F32, tag=f"ctx{hh}",
                                     name=f"ctx{p}_{hh}")
                            for hh in range(2)]

                def ctx_mms_tt(p, ex, ps_cs, tt):
                    for hh in range(2):
                        for ch in range(2):
                            nc.tensor.matmul(
                                ps_cs[hh][:, ch * 512:(ch + 1) * 512],
                                vm[:, tt, 2 * p + hh, :],
                                ex[:, tt, hh, ch, :],
                                start=(tt == 0), stop=(tt == ST - 1),
                            )

                def ctx_norm(p, ps_cs):
                    rc0 = small.tile([1, S], BF16, tag="recip0")
                    rc1 = small.tile([1, S], BF16, tag="recip1")
                    nc.vector.reciprocal(rc0[:], ps_cs[0][64:65, :])
                    nc.vector.reciprocal(rc1[:], ps_cs[1][64:65, :])
                    cu0 = small.tile([64, S], BF16, tag="cun0")
                    cu1 = small.tile([64, S], BF16, tag="cun1")
                    nc.scalar.copy(cu0[:], ps_cs[0][0:64, :])
                    nc.vector.tensor_copy(cu1[:], ps_cs[1][0:64, :])
                    bounce = drb.tile([2, S], BF16)
                    rb = rbp.tile([64, 2, S], BF16, tag="rb")
                    nc.sync.dma_start(bounce[0:1, :], rc0[:])
                    nc.sync.dma_start(
                        rb[:, 0, :], bounce[0:1, :].broadcast_to([64, S]))
                    nc.gpsimd.dma_start(bounce[1:2, :], rc1[:])
                    nc.gpsimd.dma_start(
                        rb[:, 1, :], bounce[1:2, :].broadcast_to([64, S]))
                    nc.vector.tensor_mul(
                        ctxn[p][0:64, :], cu0[:], rb[:, 0, :])
                    nc.vector.tensor_mul(
                        ctxn[p][64:128, :], cu1[:], rb[:, 1, :])

                def ctx_pair(p, ex, psC):
                    ps_cs = ctx_alloc(p, psC)
                    for tt in range(ST):
                        ctx_mms_tt(p, ex, ps_cs, tt)
                    ctx_norm(p, ps_cs)

                # ================= seg 1: qk pairs 0,1 =================
                with (
                    tc.tile_pool(name="psA2", bufs=1, space="PSUM") as psA2,
                    tc.tile_pool(name="psSh", bufs=1, space="PSUM") as psSh,
                ):
                    seg1_ps = {}
                    for i, m in enumerate((4, 0, 5, 1)):
                        # qk psums borrow the scores-pool tags (viewed flat)
                        if i % 2 == 0:
                            ps_q = sc_tile(f"qkps{m}", f"sc{i // 2}")
                        else:
                            ps_q = psA2.tile([128, 2, 512], F32, tag="qk",
                                             name=f"qkps{m}")
                        fl = ps_q[:].rearrange("p a b -> p (a b)")
                        qk_unit_mms(m, fl)
                        qk_unit_tail(m, fl, psSh)

                    # ============ w0: stream pair 0 + qk pairs 2,3 =========
                    ps_sc = (sc_tile("sc0_0", "sc0"), sc_tile("sc1_0", "sc1"))
                    ex0 = new_ex(0)
                    for tt in range(ST):
                        scores_tt(0, tt, ps_sc, ex0)
                        m = (6, 2, 7, 3)[tt // 2]
                        if tt % 2 == 0:
                            psq = psA2.tile([128, 2, 512], F32, tag="qk",
                                            name=f"qkps{m}")
                        qk_unit_mms_half(m, psq[:].rearrange(
                            "p a b -> p (a b)"), tt % 2)
                        if tt % 2 == 1:
                            qk_unit_tail(m, psq[:].rearrange(
                                "p a b -> p (a b)"), psSh)

                # ============ w1: stream pair 1 + v projection =============
                with tc.tile_pool(name="psV", bufs=1, space="PSUM") as psV:
                    ps_sc = (sc_tile("sc0_1", "sc0"), sc_tile("sc1_1", "sc1"))
                    ex1 = new_ex(1)
                    for tt in range(ST):
                        scores_tt(1, tt, ps_sc, ex1)
                        vps = psV.tile([128, 512], F32, tag=f"v{tt % 2}",
                                       name=f"vps{tt}")
                        for kt in range(KT):
                            nc.tensor.matmul(
                                vps[:],
                                xT_sb[:, kt, tt * 128:(tt + 1) * 128],
                                wv_sb[:, kt, :],
                                start=(kt == 0), stop=(kt == KT - 1),
                            )
                        nc.scalar.copy(
                            vm[:, tt, :, 0:64],
                            vps[:].rearrange("p (h d) -> p h d", d=64))

                # ===== w2, w3: stream pairs 2,3 + ctx pairs 0,1,2 ==========
                with tc.tile_pool(name="psC", bufs=1, space="PSUM") as psC:
                    ps_sc = (sc_tile("sc0_2", "sc0"), sc_tile("sc1_2", "sc1"))
                    ex2 = new_ex(2)
                    ps_cs0 = ctx_alloc(0, psC)
                    for tt in range(ST):
                        scores_tt(2, tt, ps_sc, ex2)
                        ctx_mms_tt(0, ex0, ps_cs0, tt)
                    ctx_norm(0, ps_cs0)

                    ps_sc = (sc_tile("sc0_3", "sc0"), sc_tile("sc1_3", "sc1"))
                    ex3 = new_ex(3)
                    ps_cs1 = ctx_alloc(1, psC)
                    for tt in range(ST):
                        scores_tt(3, tt, ps_sc, ex3)
                        ctx_mms_tt(1, ex1, ps_cs1, tt)
                    ctx_norm(1, ps_cs1)
                    ctx_pair(2, ex2, psC)

                    # ================= tail: ctx 3 + out-projection ========
                    ctx_pair(3, ex3, psC)

                    for grp in range(4):
                        mg = range(grp * 2, grp * 2 + 2)
                        psy = {m: sc_tile(f"psy{m}", f"sc{i}")[:].rearrange(
                            "p a b -> p (a b)") for i, m in enumerate(mg)}
                        for kt in range(4):
                            for m in mg:
                                for ch in range(2):
                                    nc.tensor.matmul(
                                        psy[m][:, ch * 512:(ch + 1) * 512],
                                        wout_sb[:, kt,
                                                m * 128:(m + 1) * 128],
                                        ctxn[kt][:,
                                                 ch * 512:(ch + 1) * 512],
                                        start=(kt == 0), stop=(kt == 3),
                                    )
                        for m in mg:
                            y_sb = tp.tile([128, S], BF16, tag="ysb",
                                           name=f"ysb{m}")
                            if m % 2 == 0:
                                nc.scalar.copy(y_sb[:], psy[m])
                            else:
                                nc.vector.tensor_copy(y_sb[:], psy[m])
                            nc.sync.dma_start(
                                yT[m * 128:(m + 1) * 128, :], y_sb[:])

    return nc


def _split_waits(nc, max_waits=1):
    """This walrus build rejects >1 sync-wait command per instruction; hoist
    extra waits onto preceding NoOps on the same engine/queue."""
    for bb in nc.main_func.blocks:
        new_insts = []
        for ins in bb.instructions:
            si = getattr(ins, "sync_info", None)
            if si is not None and si.on_wait and len(si.on_wait) > max_waits:
                waits = list(si.on_wait)
                head, rest = waits[:max_waits], waits[max_waits:]
                while rest:
                    chunk, rest = rest[:max_waits], rest[max_waits:]
                    new_insts.append(mybir.InstNoOp(
                        name=f"waitsplit-{nc.next_id()}", ins=[], outs=[],
                        sync_info=mybir.SyncInfo(on_wait=chunk, on_update=[]),
                        engine=ins.engine))
                ins.sync_info = mybir.SyncInfo(
                    on_wait=head, on_update=list(si.on_update or []))
            new_insts.append(ins)
        bb.instructions = new_insts


def make_core_inputs(x, attention_mask, Wqkv, bqkv, Wout):
    """Host-side shard prep: returns list of 8 in_maps (core = 2*b + g)."""
    Wr = np.ascontiguousarray(Wqkv).reshape(HID, 3, H, D)
    br = np.ascontiguousarray(bqkv).reshape(3, H, D)

    inv = 1.0 / (THETA ** (np.arange(0, D, 2, dtype=np.float64) / D))
    pos = np.arange(S, dtype=np.float64)
    freqs = pos[:, None] * inv[None, :]              # [S, 32]
    emb = np.concatenate([freqs, freqs], axis=1)     # [S, 64]
    cosT = np.cos(emb).T.astype(np.float32)          # [64, S]
    sgn = np.concatenate([-np.ones(32), np.ones(32)])[:, None]
    sinTs = (sgn * np.sin(emb).T).astype(np.float32)
    cos2 = np.concatenate([cosT, cosT], 0)           # [128, S]
    sin2 = np.concatenate([sinTs, sinTs], 0)

    pp = np.arange(128)
    shmap = (pp - pp % 64) + (pp % 64 + 32) % 64
    permT = np.zeros((128, 128), dtype=np.float32)
    permT[shmap, pp] = 1.0

    in_maps = []
    for c in range(NCORES):
        b, g = c // 2, c % 2
        # wqk grouped per head pair: [pair, HID, (q 128 | k 128)]
        wqk_ = np.empty((4, HID, 256), dtype=ml_dtypes.bfloat16)
        for p_ in range(4):
            hs = slice(g * HPC + 2 * p_, g * HPC + 2 * p_ + 2)
            wqk_[p_, :, 0:128] = Wr[:, 0, hs, :].reshape(HID, 128)
            wqk_[p_, :, 128:256] = Wr[:, 1, hs, :].reshape(HID, 128)
        hsg = slice(g * HPC, (g + 1) * HPC)
        wv_ = Wr[:, 2, hsg, :].reshape(HID, 512)
        bqk_ = np.concatenate(
            [br[0, hsg].reshape(512), br[1, hsg].reshape(512)]
        ).reshape(8, 128).T
        bqksh_ = bqk_[shmap]
        mb = (NEGB * (1.0 - attention_mask[b].astype(np.float32))
              ).reshape(ST, 128).T
        in_maps.append({
            "xT": np.ascontiguousarray(x[b].T.astype(ml_dtypes.bfloat16)),
            "wqk": np.ascontiguousarray(wqk_),
            "wv": np.ascontiguousarray(wv_.astype(ml_dtypes.bfloat16)),
            "bqk": np.ascontiguousarray(bqk_.astype(np.float32)),
            "bqksh": np.ascontiguousarray(bqksh_.astype(np.float32)),
            "permT": permT.astype(ml_dtypes.bfloat16),
            "cosT": cos2.astype(ml_dtypes.bfloat16),
            "sinT": sin2.astype(ml_dtypes.bfloat16),
            "mbias": np.ascontiguousarray(mb.astype(np.float32)),
            "wout": np.ascontiguousarray(
                Wout[g * 512:(g + 1) * 512, :].astype(ml_dtypes.bfloat16)),
        })
    return in_maps


_PROGRAM = None


def kernel(x, attention_mask, Wqkv, bqkv, Wout, bout, _trace=False):
    global _PROGRAM
    x = np.asarray(x)
    attention_mask = np.asarray(attention_mask)
    Wqkv = np.asarray(Wqkv)
    bqkv = np.asarray(bqkv)
    Wout = np.asarray(Wout)
    bout = np.asarray(bout)

    if _PROGRAM is None:
        _PROGRAM = build_program()
        _split_waits(_PROGRAM)
    nc = _PROGRAM

    in_maps = make_core_inputs(x, attention_mask, Wqkv, bqkv, Wout)
    res = run_bass_kernel_spmd(
        nc, in_maps, core_ids=list(range(NCORES)), trace=_trace)

    y = np.empty((B, S, HID), dtype=np.float32)
    for b in range(B):
        acc = (res.results[2 * b]["yT"].astype(np.float32)
               + res.results[2 * b + 1]["yT"].astype(np.float32))
        y[b] = acc.T
    # exact host-side bias corrections: v-bias shifts context by a constant
    # (attn rows sum to 1), q/k biases were applied on device.
    bv = bqkv[2 * HID:3 * HID].astype(np.float32)
    y += (bv @ Wout + bout).astype(np.float32)[None, None, :]
    if _trace:
        kernel.last_exec_time_ns = res.exec_time_ns
    return y


# revision 4
# speedup vs baseline: 1.0312x; 1.0229x over previous
"""DeBERTa-RoPE self-attention on 8 Trainium2 cores — v4 (bf16 streaming).

Sharding: data-parallel over batch (4) x tensor-parallel over heads (2 groups
of 8); core = 2*b + g. Host sums the two row-parallel out-proj partials.

All matmul paths run in bf16 (f32 PSUM accumulation); fp8 was measured to
break the 2e-2 gate. The schedule keeps the PE dense end-to-end:

  seg1: qk-proj pairs 0,1 (psum = scores-pool tiles viewed flat)
  w0:   scores+exp pair 0   | qk-proj pairs 2,3 (single rotating bank-pair)
  w1:   scores+exp pair 1   | v-projection (2-bank rolling)
  w2:   scores+exp pair 2   | ctx pair 0 (+softmax-normalize)
  w3:   scores+exp pair 3   | ctx pairs 1,2
  tail: ctx pair 3, out-projection (psum = scores-pool tiles again)

The attention mask folds into the exp bias (masked keys exp to ~0); the
denominator rides as a 65th stationary column of v. exps live in SBUF bf16,
two pair-sized buffers rotating.
"""

import numpy as np
import ml_dtypes

import concourse.bass as bass
import concourse.mybir as mybir
import concourse.tile as tile
from concourse.bass_utils import run_bass_kernel_spmd

H = 16
D = 64
HID = H * D
B = 4
S = 1024
THETA = 10000.0
NCORES = 8
HPC = H // 2          # heads per core
KT = HID // 128       # 8 k-tiles
ST = S // 128         # 8 seq tiles

F32 = mybir.dt.float32
F32R = mybir.dt.float32r
BF16 = mybir.dt.bfloat16
AF = mybir.ActivationFunctionType
ALU = mybir.AluOpType

NEGB = -30.0          # bias for masked keys: exp(s*0.125 - 30) ~ 0


def _r(ap):
    return ap.bitcast(F32R)


def build_program():
    nc = bass.Bass()
    xT = nc.declare_dram_parameter("xT", [HID, S], BF16, isOutput=False)
    wqk = nc.declare_dram_parameter("wqk", [4, HID, 256], BF16, isOutput=False)
    wv = nc.declare_dram_parameter("wv", [HID, 512], BF16, isOutput=False)
    bqk = nc.declare_dram_parameter("bqk", [128, 8], F32, isOutput=False)
    bqksh = nc.declare_dram_parameter("bqksh", [128, 8], F32, isOutput=False)
    cosT = nc.declare_dram_parameter("cosT", [128, S], BF16, isOutput=False)
    sinT = nc.declare_dram_parameter("sinT", [128, S], BF16, isOutput=False)
    mbias = nc.declare_dram_parameter("mbias", [128, ST], F32, isOutput=False)
    wout = nc.declare_dram_parameter("wout", [512, HID], BF16, isOutput=False)
    permT = nc.declare_dram_parameter("permT", [128, 128], BF16, isOutput=False)
    yT = nc.declare_dram_parameter("yT", [HID, S], BF16, isOutput=True)

    with tile.TileContext(nc) as tc, nc.allow_low_precision(
            reason="bf16 recip/cos tables; rel-err budget 2e-2"):
        with (
            tc.tile_pool(name="const", bufs=1) as cpool,
            tc.tile_pool(name="persist", bufs=1) as persist,
        ):
            cos_sb = cpool.tile([128, S], BF16)
            sin_sb = cpool.tile([128, S], BF16)
            mbias_sb = cpool.tile([128, ST], F32)
            bqk_sb = cpool.tile([128, 8], F32)
            bqksh_sb = cpool.tile([128, 8], F32)
            permT_sb = cpool.tile([128, 128], BF16)

            rope = [persist.tile([128, S], BF16, name=f"rope{i}")
                    for i in range(8)]
            # v + denominator-ones column: [t, tt, head, 65]
            vm = persist.tile([128, ST, HPC, 65], BF16)
            ctxn = [persist.tile([128, S], BF16, name=f"ctxn{i}")
                    for i in range(4)]
            xT_sb = persist.tile([128, KT, S], BF16)
            wqk_sb = persist.tile([128, KT, 4, 256], BF16)
            wv_sb = persist.tile([128, KT, 512], BF16)
            wout_sb = persist.tile([128, 4, HID], BF16)

            # ---- input DMAs ----
            nc.scalar.dma_start(cos_sb[:], cosT[:])
            nc.scalar.dma_start(sin_sb[:], sinT[:])
            nc.scalar.dma_start(mbias_sb[:], mbias[:])
            nc.scalar.dma_start(bqk_sb[:], bqk[:])
            nc.scalar.dma_start(bqksh_sb[:], bqksh[:])
            nc.scalar.dma_start(permT_sb[:], permT[:])
            nc.sync.dma_start(
                wqk_sb[:, :, 0, :],
                wqk[0].rearrange("(k p) n -> p k n", p=128))
            for c in range(4):
                nc.sync.dma_start(
                    xT_sb[:, 2 * c:2 * c + 2, :],
                    xT[c * 256:(c + 1) * 256, :].rearrange(
                        "(k p) s -> p k s", p=128))
            nc.sync.dma_start(
                wqk_sb[:, :, 1, :],
                wqk[1].rearrange("(k p) n -> p k n", p=128))
            nc.scalar.dma_start(
                wv_sb[:], wv[:].rearrange("(k p) n -> p k n", p=128))
            for q_ in (2, 3):
                nc.sync.dma_start(
                    wqk_sb[:, :, q_, :],
                    wqk[q_].rearrange("(k p) n -> p k n", p=128))
            nc.sync.dma_start(
                wout_sb[:], wout[:].rearrange("(k p) n -> p k n", p=128))

            # denominator ones-column
            nc.gpsimd.memset(vm[:, :, :, 64:65], 1.0)

            with (
                tc.tile_pool(name="qksb", bufs=1) as qkp,
                tc.tile_pool(name="ropetmp", bufs=1) as rt,
                tc.tile_pool(name="expool", bufs=1) as exp_pool,
                tc.tile_pool(name="tailp", bufs=2) as tp,
                tc.tile_pool(name="small", bufs=1) as small,
                tc.tile_pool(name="rbp", bufs=1) as rbp,
                tc.tile_pool(name="drbounce", bufs=2, space="DRAM") as drb,
                tc.tile_pool(name="psS", bufs=1, space="PSUM") as psS,
            ):
                def sc_tile(name_, tag):
                    return psS.tile([128, 2, 512], F32, tag=tag, name=name_)

                def qk_unit_mms_half(m, ps_qk, half):
                    for kt in range(4 * half, 4 * half + 4):
                        for ch in range(2):
                            nc.tensor.matmul(
                                ps_qk[:, ch * 512:(ch + 1) * 512],
                                wqk_sb[:, kt, m % 4, (m // 4) * 128:
                                       (m // 4) * 128 + 128],
                                xT_sb[:, kt, ch * 512:(ch + 1) * 512],
                                start=(kt == 0), stop=(kt == KT - 1),
                            )

                def qk_unit_mms(m, ps_qk):
                    for kt in range(KT):
                        for ch in range(2):
                            nc.tensor.matmul(
                                ps_qk[:, ch * 512:(ch + 1) * 512],
                                wqk_sb[:, kt, m % 4, (m // 4) * 128:
                                       (m // 4) * 128 + 128],
                                xT_sb[:, kt, ch * 512:(ch + 1) * 512],
                                start=(kt == 0), stop=(kt == KT - 1),
                            )

                def qk_unit_tail(m, ps_qk, psSh):
                    qk_sb = qkp.tile([128, S], BF16, tag="qksb",
                                     name=f"qksb{m}")
                    nc.vector.tensor_copy(qk_sb[:], ps_qk[:])
                    ps_sh = psSh.tile([128, S], F32, tag="sh",
                                      name=f"pssh{m}")
                    for ch in range(2):
                        nc.tensor.matmul(
                            ps_sh[:, ch * 512:(ch + 1) * 512],
                            permT_sb[:],
                            qk_sb[:, ch * 512:(ch + 1) * 512],
                            start=True, stop=True,
                        )
                    for ch in range(2):
                        sl = slice(ch * 512, (ch + 1) * 512)
                        t1 = rt.tile([128, 512], BF16, tag=f"t1{ch}")
                        nc.vector.scalar_tensor_tensor(
                            t1[:], ps_qk[:, sl], bqk_sb[:, m:m + 1],
                            cos_sb[:, sl], op0=ALU.add, op1=ALU.mult)
                        s2 = rt.tile([128, 512], BF16, tag=f"s2{ch}")
                        nc.vector.scalar_tensor_tensor(
                            s2[:], ps_sh[:, sl], bqksh_sb[:, m:m + 1],
                            sin_sb[:, sl], op0=ALU.add, op1=ALU.mult)
                        nc.vector.tensor_add(
                            rope[m][:, sl], t1[:], s2[:])

                def scores_tt(p, tt, ps_sc, ex):
                    qp = rope[p][:]
                    kp = rope[p + 4][:]
                    for ch in range(2):
                        for hh in range(2):
                            base = hh * 64
                            nc.tensor.matmul(
                                ps_sc[ch][:, hh, :],
                                kp[base:base + 64, tt * 128:(tt + 1) * 128],
                                qp[base:base + 64, ch * 512:(ch + 1) * 512],
                                start=True, stop=True,
                                tile_position=(base, 0),
                            )
                        nc.scalar.activation(
                            ex[:, tt, :, ch, :], ps_sc[ch][:],
                            AF.Exp, bias=mbias_sb[:, tt:tt + 1], scale=0.125)

                def new_ex(p):
                    # exps for one pair: [t, tt, hh, ch, s]; 2 rotating bufs
                    return exp_pool.tile([128, ST, 2, 2, 512], BF16,
                                         tag=f"ex{p % 3}", name=f"ex{p}")

                def ctx_alloc(p, psC):
                    return [psC.tile([65, S], F32, tag=f"ctx{hh}",
                                     name=f"ctx{p}_{hh}")
                            for hh in range(2)]

                def ctx_mms_tt(p, ex, ps_cs, tt):
                    for hh in range(2):
                        for ch in range(2):
                            nc.tensor.matmul(
                                ps_cs[hh][:, ch * 512:(ch + 1) * 512],
                                vm[:, tt, 2 * p + hh, :],
                                ex[:, tt, hh, ch, :],
                                start=(tt == 0), stop=(tt == ST - 1),
                            )

                def ctx_norm(p, ps_cs):
                    rc0 = small.tile([1, S], BF16, tag="recip0")
                    rc1 = small.tile([1, S], BF16, tag="recip1")
                    nc.vector.reciprocal(rc0[:], ps_cs[0][64:65, :])
                    nc.vector.reciprocal(rc1[:], ps_cs[1][64:65, :])
                    cu0 = small.tile([64, S], BF16, tag="cun0")
                    cu1 = small.tile([64, S], BF16, tag="cun1")
                    nc.scalar.copy(cu0[:], ps_cs[0][0:64, :])
                    nc.vector.tensor_copy(cu1[:], ps_cs[1][0:64, :])
                    bounce = drb.tile([2, S], BF16)
                    rb = rbp.tile([64, 2, S], BF16, tag="rb")
                    nc.sync.dma_start(bounce[0:1, :], rc0[:])
                    nc.sync.dma_start(
                        rb[:, 0, :], bounce[0:1, :].broadcast_to([64, S]))
                    nc.gpsimd.dma_start(bounce[1:2, :], rc1[:])
                    nc.gpsimd.dma_start(
                        rb[:, 1, :], bounce[1:2, :].broadcast_to([64, S]))
                    nc.vector.tensor_mul(
                        ctxn[p][0:64, :], cu0[:], rb[:, 0, :])
                    nc.vector.tensor_mul(
                        ctxn[p][64:128, :], cu1[:], rb[:, 1, :])

                def ctx_pair(p, ex, psC):
                    ps_cs = ctx_alloc(p, psC)
                    for tt in range(ST):
                        ctx_mms_tt(p, ex, ps_cs, tt)
                    ctx_norm(p, ps_cs)

                # ================= seg 1: qk pairs 0,1 =================
                with (
                    tc.tile_pool(name="psA2", bufs=1, space="PSUM") as psA2,
                    tc.tile_pool(name="psSh", bufs=1, space="PSUM") as psSh,
                ):
                    seg1_ps = {}
                    for i, m in enumerate((4, 0, 5, 1)):
                        # qk psums borrow the scores-pool tags (viewed flat)
                        if i % 2 == 0:
                            ps_q = sc_tile(f"qkps{m}", f"sc{i // 2}")
                        else:
                            ps_q = psA2.tile([128, 2, 512], F32, tag="qk",
                                             name=f"qkps{m}")
                        fl = ps_q[:].rearrange("p a b -> p (a b)")
                        qk_unit_mms(m, fl)
                        qk_unit_tail(m, fl, psSh)

                    # ============ w0: stream pair 0 + qk pairs 2,3 =========
                    ps_sc = (sc_tile("sc0_0", "sc0"), sc_tile("sc1_0", "sc1"))
                    ex0 = new_ex(0)
                    for tt in range(ST):
                        scores_tt(0, tt, ps_sc, ex0)
                        m = (6, 2, 7, 3)[tt // 2]
                        if tt % 2 == 0:
                            psq = psA2.tile([128, 2, 512], F32, tag="qk",
                                            name=f"qkps{m}")
                        qk_unit_mms_half(m, psq[:].rearrange(
                            "p a b -> p (a b)"), tt % 2)
                        if tt % 2 == 1:
                            qk_unit_tail(m, psq[:].rearrange(
                                "p a b -> p (a b)"), psSh)

                # ============ w1: stream pair 1 + v projection =============
                with tc.tile_pool(name="psV", bufs=1, space="PSUM") as psV:
                    ps_sc = (sc_tile("sc0_1", "sc0"), sc_tile("sc1_1", "sc1"))
                    ex1 = new_ex(1)
                    for tt in range(ST):
                        scores_tt(1, tt, ps_sc, ex1)
                        vps = psV.tile([128, 512], F32, tag=f"v{tt % 2}",
                                       name=f"vps{tt}")
                        for kt in range(KT):
                            nc.tensor.matmul(
                                vps[:],
                                xT_sb[:, kt, tt * 128:(tt + 1) * 128],
                                wv_sb[:, kt, :],
                                start=(kt == 0), stop=(kt == KT - 1),
                            )
                        nc.scalar.copy(
                            vm[:, tt, :, 0:64],
                            vps[:].rearrange("p (h d) -> p h d", d=64))

                # ===== w2, w3: stream pairs 2,3 + ctx pairs 0,1,2 ==========
                with tc.tile_pool(name="psC", bufs=1, space="PSUM") as psC:
                    ps_sc = (sc_tile("sc0_2", "sc0"), sc_tile("sc1_2", "sc1"))
                    ex2 = new_ex(2)
                    ps_cs0 = ctx_alloc(0, psC)
                    for tt in range(ST):
                        scores_tt(2, tt, ps_sc, ex2)
                        ctx_mms_tt(0, ex0, ps_cs0, tt)
                    ctx_norm(0, ps_cs0)

                    ps_sc = (sc_tile("sc0_3", "sc0"), sc_tile("sc1_3", "sc1"))
                    ex3 = new_ex(3)
                    ps_cs1 = ctx_alloc(1, psC)
                    for tt in range(ST):
                        scores_tt(3, tt, ps_sc, ex3)
                        ctx_mms_tt(1, ex1, ps_cs1, tt)
                    ctx_norm(1, ps_cs1)
                    ctx_pair(2, ex2, psC)

                    # ================= tail: ctx 3 + out-projection ========
                    ctx_pair(3, ex3, psC)

                    def d_mms(psy, mg, kts):
                        for kt in kts:
                            for m in mg:
                                for ch in range(2):
                                    nc.tensor.matmul(
                                        psy[m][:, ch * 512:(ch + 1) * 512],
                                        wout_sb[:, kt,
                                                m * 128:(m + 1) * 128],
                                        ctxn[kt][:,
                                                 ch * 512:(ch + 1) * 512],
                                        start=(kt == 0), stop=(kt == 3),
                                    )

                    def d_psy(grp):
                        mg = list(range(grp * 2, grp * 2 + 2))
                        if grp % 2 == 0:
                            return mg, {m: sc_tile(f"psy{m}", f"sc{i}")[:]
                                        .rearrange("p a b -> p (a b)")
                                        for i, m in enumerate(mg)}
                        # odd groups borrow the freed psC banks
                        return mg, {m: psC.tile([128, 2, 512], F32,
                                                tag=f"ctx{i}",
                                                name=f"psy{m}")[:]
                                    .rearrange("p a b -> p (a b)")
                                    for i, m in enumerate(mg)}

                    def d_evac(psy, mg):
                        for m in mg:
                            y_sb = tp.tile([128, S], BF16, tag="ysb",
                                           name=f"ysb{m}")
                            if m % 2 == 0:
                                nc.scalar.copy(y_sb[:], psy[m])
                            else:
                                nc.vector.tensor_copy(y_sb[:], psy[m])
                            nc.sync.dma_start(
                                yT[m * 128:(m + 1) * 128, :], y_sb[:])

                    # fill the norm(3) latency: groups 0 and 1 run their
                    # ctxn0-2 slices first, the final slice after the norm
                    mg0, psy0 = d_psy(0)
                    d_mms(psy0, mg0, (0, 1, 2))
                    mg1, psy1 = d_psy(1)
                    d_mms(psy1, mg1, (0, 1, 2))
                    d_mms(psy0, mg0, (3,))
                    d_evac(psy0, mg0)
                    d_mms(psy1, mg1, (3,))
                    d_evac(psy1, mg1)
                    for grp in (2, 3):
                        mg, psy = d_psy(grp)
                        d_mms(psy, mg, range(4))
                        d_evac(psy, mg)

    return nc


def _split_waits(nc, max_waits=1):
    """This walrus build rejects >1 sync-wait command per instruction; hoist
    extra waits onto preceding NoOps on the same engine/queue."""
    for bb in nc.main_func.blocks:
        new_insts = []
        for ins in bb.instructions:
            si = getattr(ins, "sync_info", None)
            if si is not None and si.on_wait and len(si.on_wait) > max_waits:
                waits = list(si.on_wait)
                head, rest = waits[:max_waits], waits[max_waits:]
                while rest:
                    chunk, rest = rest[:max_waits], rest[max_waits:]
                    new_insts.append(mybir.InstNoOp(
                        name=f"waitsplit-{nc.next_id()}", ins=[], outs=[],
                        sync_info=mybir.SyncInfo(on_wait=chunk, on_update=[]),
                        engine=ins.engine))
                ins.sync_info = mybir.SyncInfo(
                    on_wait=head, on_update=list(si.on_update or []))
            new_insts.append(ins)
        bb.instructions = new_insts


def make_core_inputs(x, attention_mask, Wqkv, bqkv, Wout):
    """Host-side shard prep: returns list of 8 in_maps (core = 2*b + g)."""
    Wr = np.ascontiguousarray(Wqkv).reshape(HID, 3, H, D)
    br = np.ascontiguousarray(bqkv).reshape(3, H, D)

    inv = 1.0 / (THETA ** (np.arange(0, D, 2, dtype=np.float64) / D))
    pos = np.arange(S, dtype=np.float64)
    freqs = pos[:, None] * inv[None, :]              # [S, 32]
    emb = np.concatenate([freqs, freqs], axis=1)     # [S, 64]
    cosT = np.cos(emb).T.astype(np.float32)          # [64, S]
    sgn = np.concatenate([-np.ones(32), np.ones(32)])[:, None]
    sinTs = (sgn * np.sin(emb).T).astype(np.float32)
    cos2 = np.concatenate([cosT, cosT], 0)           # [128, S]
    sin2 = np.concatenate([sinTs, sinTs], 0)

    pp = np.arange(128)
    shmap = (pp - pp % 64) + (pp % 64 + 32) % 64
    permT = np.zeros((128, 128), dtype=np.float32)
    permT[shmap, pp] = 1.0

    in_maps = []
    for c in range(NCORES):
        b, g = c // 2, c % 2
        # wqk grouped per head pair: [pair, HID, (q 128 | k 128)]
        wqk_ = np.empty((4, HID, 256), dtype=ml_dtypes.bfloat16)
        for p_ in range(4):
            hs = slice(g * HPC + 2 * p_, g * HPC + 2 * p_ + 2)
            wqk_[p_, :, 0:128] = Wr[:, 0, hs, :].reshape(HID, 128)
            wqk_[p_, :, 128:256] = Wr[:, 1, hs, :].reshape(HID, 128)
        hsg = slice(g * HPC, (g + 1) * HPC)
        wv_ = Wr[:, 2, hsg, :].reshape(HID, 512)
        bqk_ = np.concatenate(
            [br[0, hsg].reshape(512), br[1, hsg].reshape(512)]
        ).reshape(8, 128).T
        bqksh_ = bqk_[shmap]
        mb = (NEGB * (1.0 - attention_mask[b].astype(np.float32))
              ).reshape(ST, 128).T
        in_maps.append({
            "xT": np.ascontiguousarray(x[b].T.astype(ml_dtypes.bfloat16)),
            "wqk": np.ascontiguousarray(wqk_),
            "wv": np.ascontiguousarray(wv_.astype(ml_dtypes.bfloat16)),
            "bqk": np.ascontiguousarray(bqk_.astype(np.float32)),
            "bqksh": np.ascontiguousarray(bqksh_.astype(np.float32)),
            "permT": permT.astype(ml_dtypes.bfloat16),
            "cosT": cos2.astype(ml_dtypes.bfloat16),
            "sinT": sin2.astype(ml_dtypes.bfloat16),
            "mbias": np.ascontiguousarray(mb.astype(np.float32)),
            "wout": np.ascontiguousarray(
                Wout[g * 512:(g + 1) * 512, :].astype(ml_dtypes.bfloat16)),
        })
    return in_maps


_PROGRAM = None


def kernel(x, attention_mask, Wqkv, bqkv, Wout, bout, _trace=False):
    global _PROGRAM
    x = np.asarray(x)
    attention_mask = np.asarray(attention_mask)
    Wqkv = np.asarray(Wqkv)
    bqkv = np.asarray(bqkv)
    Wout = np.asarray(Wout)
    bout = np.asarray(bout)

    if _PROGRAM is None:
        _PROGRAM = build_program()
        _split_waits(_PROGRAM)
    nc = _PROGRAM

    in_maps = make_core_inputs(x, attention_mask, Wqkv, bqkv, Wout)
    res = run_bass_kernel_spmd(
        nc, in_maps, core_ids=list(range(NCORES)), trace=_trace)

    y = np.empty((B, S, HID), dtype=np.float32)
    for b in range(B):
        acc = (res.results[2 * b]["yT"].astype(np.float32)
               + res.results[2 * b + 1]["yT"].astype(np.float32))
        y[b] = acc.T
    # exact host-side bias corrections: v-bias shifts context by a constant
    # (attn rows sum to 1), q/k biases were applied on device.
    bv = bqkv[2 * HID:3 * HID].astype(np.float32)
    y += (bv @ Wout + bout).astype(np.float32)[None, None, :]
    if _trace:
        kernel.last_exec_time_ns = res.exec_time_ns
    return y


# revision 5
# speedup vs baseline: 1.0356x; 1.0042x over previous
"""DeBERTa-RoPE self-attention on 8 Trainium2 cores — v4 (bf16 streaming).

Sharding: data-parallel over batch (4) x tensor-parallel over heads (2 groups
of 8); core = 2*b + g. Host sums the two row-parallel out-proj partials.

All matmul paths run in bf16 (f32 PSUM accumulation); fp8 was measured to
break the 2e-2 gate. The schedule keeps the PE dense end-to-end:

  seg1: qk-proj pairs 0,1 (psum = scores-pool tiles viewed flat)
  w0:   scores+exp pair 0   | qk-proj pairs 2,3 (single rotating bank-pair)
  w1:   scores+exp pair 1   | v-projection (2-bank rolling)
  w2:   scores+exp pair 2   | ctx pair 0 (+softmax-normalize)
  w3:   scores+exp pair 3   | ctx pairs 1,2
  tail: ctx pair 3, out-projection (psum = scores-pool tiles again)

The attention mask folds into the exp bias (masked keys exp to ~0); the
denominator rides as a 65th stationary column of v. exps live in SBUF bf16,
two pair-sized buffers rotating.
"""

import numpy as np
import ml_dtypes

import concourse.bass as bass
import concourse.mybir as mybir
import concourse.tile as tile
from concourse.bass_utils import run_bass_kernel_spmd

H = 16
D = 64
HID = H * D
B = 4
S = 1024
THETA = 10000.0
NCORES = 8
HPC = H // 2          # heads per core
KT = HID // 128       # 8 k-tiles
ST = S // 128         # 8 seq tiles

F32 = mybir.dt.float32
F32R = mybir.dt.float32r
BF16 = mybir.dt.bfloat16
AF = mybir.ActivationFunctionType
ALU = mybir.AluOpType

NEGB = -30.0          # bias for masked keys: exp(s*0.125 - 30) ~ 0


def _r(ap):
    return ap.bitcast(F32R)


def build_program():
    nc = bass.Bass()
    xT = nc.declare_dram_parameter("xT", [HID, S], BF16, isOutput=False)
    wqk = nc.declare_dram_parameter("wqk", [4, HID, 256], BF16, isOutput=False)
    wv = nc.declare_dram_parameter("wv", [HID, 512], BF16, isOutput=False)
    bqk = nc.declare_dram_parameter("bqk", [128, 8], F32, isOutput=False)
    bqksh = nc.declare_dram_parameter("bqksh", [128, 8], F32, isOutput=False)
    cosT = nc.declare_dram_parameter("cosT", [128, S], BF16, isOutput=False)
    sinT = nc.declare_dram_parameter("sinT", [128, S], BF16, isOutput=False)
    mbias = nc.declare_dram_parameter("mbias", [128, ST], F32, isOutput=False)
    wout = nc.declare_dram_parameter("wout", [512, HID], BF16, isOutput=False)
    permT = nc.declare_dram_parameter("permT", [128, 128], BF16, isOutput=False)
    yT = nc.declare_dram_parameter("yT", [HID, S], BF16, isOutput=True)

    with tile.TileContext(nc) as tc, nc.allow_low_precision(
            reason="bf16 recip/cos tables; rel-err budget 2e-2"):
        with (
            tc.tile_pool(name="const", bufs=1) as cpool,
            tc.tile_pool(name="persist", bufs=1) as persist,
        ):
            cos_sb = cpool.tile([128, S], BF16)
            sin_sb = cpool.tile([128, S], BF16)
            mbias_sb = cpool.tile([128, ST], F32)
            bqk_sb = cpool.tile([128, 8], F32)
            bqksh_sb = cpool.tile([128, 8], F32)
            permT_sb = cpool.tile([128, 128], BF16)

            rope = [persist.tile([128, S], BF16, name=f"rope{i}")
                    for i in range(8)]
            # v + denominator-ones column: [t, tt, head, 65]
            vm = persist.tile([128, ST, HPC, 65], BF16)
            ctxn = [persist.tile([128, S], BF16, name=f"ctxn{i}")
                    for i in range(4)]
            xT_sb = persist.tile([128, KT, S], BF16)
            wqk_sb = persist.tile([128, KT, 4, 256], BF16)
            wv_sb = persist.tile([128, KT, 512], BF16)
            wout_sb = persist.tile([128, 4, HID], BF16)

            # ---- input DMAs ----
            nc.scalar.dma_start(cos_sb[:], cosT[:])
            nc.scalar.dma_start(sin_sb[:], sinT[:])
            nc.scalar.dma_start(mbias_sb[:], mbias[:])
            nc.scalar.dma_start(bqk_sb[:], bqk[:])
            nc.scalar.dma_start(bqksh_sb[:], bqksh[:])
            nc.scalar.dma_start(permT_sb[:], permT[:])
            nc.sync.dma_start(
                wqk_sb[:, :, 0, :],
                wqk[0].rearrange("(k p) n -> p k n", p=128))
            for c in range(4):
                nc.sync.dma_start(
                    xT_sb[:, 2 * c:2 * c + 2, :],
                    xT[c * 256:(c + 1) * 256, :].rearrange(
                        "(k p) s -> p k s", p=128))
            nc.sync.dma_start(
                wqk_sb[:, :, 1, :],
                wqk[1].rearrange("(k p) n -> p k n", p=128))
            nc.scalar.dma_start(
                wv_sb[:], wv[:].rearrange("(k p) n -> p k n", p=128))
            for q_ in (2, 3):
                nc.sync.dma_start(
                    wqk_sb[:, :, q_, :],
                    wqk[q_].rearrange("(k p) n -> p k n", p=128))
            nc.sync.dma_start(
                wout_sb[:], wout[:].rearrange("(k p) n -> p k n", p=128))

            # denominator ones-column
            nc.gpsimd.memset(vm[:, :, :, 64:65], 1.0)

            with (
                tc.tile_pool(name="qksb", bufs=1) as qkp,
                tc.tile_pool(name="ropetmp", bufs=1) as rt,
                tc.tile_pool(name="expool", bufs=1) as exp_pool,
                tc.tile_pool(name="tailp", bufs=2) as tp,
                tc.tile_pool(name="small", bufs=1) as small,
                tc.tile_pool(name="rbp", bufs=1) as rbp,
                tc.tile_pool(name="drbounce", bufs=2, space="DRAM") as drb,
                tc.tile_pool(name="psS", bufs=1, space="PSUM") as psS,
            ):
                def sc_tile(name_, tag):
                    return psS.tile([128, 2, 512], F32, tag=tag, name=name_)

                def qk_unit_mms_half(m, ps_qk, half):
                    for kt in range(4 * half, 4 * half + 4):
                        for ch in range(2):
                            nc.tensor.matmul(
                                ps_qk[:, ch * 512:(ch + 1) * 512],
                                wqk_sb[:, kt, m % 4, (m // 4) * 128:
                                       (m // 4) * 128 + 128],
                                xT_sb[:, kt, ch * 512:(ch + 1) * 512],
                                start=(kt == 0), stop=(kt == KT - 1),
                            )

                def qk_unit_mms(m, ps_qk):
                    for kt in range(KT):
                        for ch in range(2):
                            nc.tensor.matmul(
                                ps_qk[:, ch * 512:(ch + 1) * 512],
                                wqk_sb[:, kt, m % 4, (m // 4) * 128:
                                       (m // 4) * 128 + 128],
                                xT_sb[:, kt, ch * 512:(ch + 1) * 512],
                                start=(kt == 0), stop=(kt == KT - 1),
                            )

                def qk_unit_tail(m, ps_qk, psSh):
                    qk_sb = qkp.tile([128, S], BF16, tag="qksb",
                                     name=f"qksb{m}")
                    nc.vector.tensor_copy(qk_sb[:], ps_qk[:])
                    ps_sh = psSh.tile([128, S], F32, tag="sh",
                                      name=f"pssh{m}")
                    for ch in range(2):
                        nc.tensor.matmul(
                            ps_sh[:, ch * 512:(ch + 1) * 512],
                            permT_sb[:],
                            qk_sb[:, ch * 512:(ch + 1) * 512],
                            start=True, stop=True,
                        )
                    for ch in range(2):
                        sl = slice(ch * 512, (ch + 1) * 512)
                        t1 = rt.tile([128, 512], BF16, tag=f"t1{ch}")
                        nc.vector.scalar_tensor_tensor(
                            t1[:], ps_qk[:, sl], bqk_sb[:, m:m + 1],
                            cos_sb[:, sl], op0=ALU.add, op1=ALU.mult)
                        s2 = rt.tile([128, 512], BF16, tag=f"s2{ch}")
                        nc.vector.scalar_tensor_tensor(
                            s2[:], ps_sh[:, sl], bqksh_sb[:, m:m + 1],
                            sin_sb[:, sl], op0=ALU.add, op1=ALU.mult)
                        nc.vector.tensor_add(
                            rope[m][:, sl], t1[:], s2[:])

                def scores_tt(p, tt, ps_sc, ex):
                    qp = rope[p][:]
                    kp = rope[p + 4][:]
                    for ch in range(2):
                        for hh in range(2):
                            base = hh * 64
                            nc.tensor.matmul(
                                ps_sc[ch][:, hh, :],
                                kp[base:base + 64, tt * 128:(tt + 1) * 128],
                                qp[base:base + 64, ch * 512:(ch + 1) * 512],
                                start=True, stop=True,
                                tile_position=(base, 0),
                            )
                        nc.scalar.activation(
                            ex[:, tt, :, ch, :], ps_sc[ch][:],
                            AF.Exp, bias=mbias_sb[:, tt:tt + 1], scale=0.125)

                def new_ex(p):
                    # exps for one pair: [t, tt, hh, ch, s]; 2 rotating bufs
                    return exp_pool.tile([128, ST, 2, 2, 512], BF16,
                                         tag=f"ex{p % 3}", name=f"ex{p}")

                def ctx_alloc(p, psC):
                    return [psC.tile([65, S], F32, tag=f"ctx{hh}",
                                     name=f"ctx{p}_{hh}")
                            for hh in range(2)]

                def ctx_mms_tt(p, ex, ps_cs, tt):
                    for hh in range(2):
                        for ch in range(2):
                            nc.tensor.matmul(
                                ps_cs[hh][:, ch * 512:(ch + 1) * 512],
                                vm[:, tt, 2 * p + hh, :],
                                ex[:, tt, hh, ch, :],
                                start=(tt == 0), stop=(tt == ST - 1),
                            )

                def ctx_norm(p, ps_cs):
                    rc0 = small.tile([1, S], BF16, tag="recip0")
                    rc1 = small.tile([1, S], BF16, tag="recip1")
                    nc.vector.reciprocal(rc0[:], ps_cs[0][64:65, :])
                    nc.vector.reciprocal(rc1[:], ps_cs[1][64:65, :])
                    cu0 = small.tile([64, S], BF16, tag="cun0")
                    cu1 = small.tile([64, S], BF16, tag="cun1")
                    nc.scalar.copy(cu0[:], ps_cs[0][0:64, :])
                    nc.vector.tensor_copy(cu1[:], ps_cs[1][0:64, :])
                    bounce = drb.tile([2, S], BF16)
                    rb = rbp.tile([64, 2, S], BF16, tag="rb")
                    nc.sync.dma_start(bounce[0:1, :], rc0[:])
                    nc.sync.dma_start(
                        rb[:, 0, :], bounce[0:1, :].broadcast_to([64, S]))
                    nc.gpsimd.dma_start(bounce[1:2, :], rc1[:])
                    nc.gpsimd.dma_start(
                        rb[:, 1, :], bounce[1:2, :].broadcast_to([64, S]))
                    nc.vector.tensor_mul(
                        ctxn[p][0:64, :], cu0[:], rb[:, 0, :])
                    nc.vector.tensor_mul(
                        ctxn[p][64:128, :], cu1[:], rb[:, 1, :])

                def ctx_pair(p, ex, psC):
                    ps_cs = ctx_alloc(p, psC)
                    for tt in range(ST):
                        ctx_mms_tt(p, ex, ps_cs, tt)
                    ctx_norm(p, ps_cs)

                # ================= seg 1: qk pairs 0,1 =================
                with (
                    tc.tile_pool(name="psA2", bufs=1, space="PSUM") as psA2,
                    tc.tile_pool(name="psSh", bufs=1, space="PSUM") as psSh,
                ):
                    seg1_ps = {}
                    for i, m in enumerate((4, 0, 5, 1)):
                        # qk psums borrow the scores-pool tags (viewed flat)
                        if i % 2 == 0:
                            ps_q = sc_tile(f"qkps{m}", f"sc{i // 2}")
                        else:
                            ps_q = psA2.tile([128, 2, 512], F32, tag="qk",
                                             name=f"qkps{m}")
                        fl = ps_q[:].rearrange("p a b -> p (a b)")
                        qk_unit_mms(m, fl)
                        qk_unit_tail(m, fl, psSh)

                    # ============ w0: stream pair 0 + qk pairs 2,3 =========
                    ps_sc = (sc_tile("sc0_0", "sc0"), sc_tile("sc1_0", "sc1"))
                    ex0 = new_ex(0)
                    for tt in range(ST):
                        scores_tt(0, tt, ps_sc, ex0)
                        m = (6, 2, 7, 3)[tt // 2]
                        if tt % 2 == 0:
                            psq = psA2.tile([128, 2, 512], F32, tag="qk",
                                            name=f"qkps{m}")
                        qk_unit_mms_half(m, psq[:].rearrange(
                            "p a b -> p (a b)"), tt % 2)
                        if tt % 2 == 1:
                            qk_unit_tail(m, psq[:].rearrange(
                                "p a b -> p (a b)"), psSh)

                # ============ w1: stream pair 1 + v projection =============
                with tc.tile_pool(name="psV", bufs=1, space="PSUM") as psV:
                    ps_sc = (sc_tile("sc0_1", "sc0"), sc_tile("sc1_1", "sc1"))
                    ex1 = new_ex(1)
                    for tt in range(ST):
                        scores_tt(1, tt, ps_sc, ex1)
                        vps = psV.tile([128, 512], F32, tag=f"v{tt % 2}",
                                       name=f"vps{tt}")
                        for kt in range(KT):
                            nc.tensor.matmul(
                                vps[:],
                                xT_sb[:, kt, tt * 128:(tt + 1) * 128],
                                wv_sb[:, kt, :],
                                start=(kt == 0), stop=(kt == KT - 1),
                            )
                        nc.scalar.copy(
                            vm[:, tt, :, 0:64],
                            vps[:].rearrange("p (h d) -> p h d", d=64))

                # ===== w2, w3: stream pairs 2,3 + ctx pairs 0,1,2 ==========
                with tc.tile_pool(name="psC", bufs=1, space="PSUM") as psC:
                    ps_sc = (sc_tile("sc0_2", "sc0"), sc_tile("sc1_2", "sc1"))
                    ex2 = new_ex(2)
                    ps_cs0 = ctx_alloc(0, psC)
                    for tt in range(ST):
                        scores_tt(2, tt, ps_sc, ex2)
                        ctx_mms_tt(0, ex0, ps_cs0, tt)
                    ctx_norm(0, ps_cs0)

                    ps_sc = (sc_tile("sc0_3", "sc0"), sc_tile("sc1_3", "sc1"))
                    ex3 = new_ex(3)
                    ps_cs1 = ctx_alloc(1, psC)
                    for tt in range(ST):
                        scores_tt(3, tt, ps_sc, ex3)
                        ctx_mms_tt(1, ex1, ps_cs1, tt)
                    ctx_norm(1, ps_cs1)
                    ctx_pair(2, ex2, psC)

                    # ================= tail: ctx 3 + out-projection ========
                    ctx_pair(3, ex3, psC)

                    def d_mms(psy, mg, kts):
                        for kt in kts:
                            for m in mg:
                                for ch in range(2):
                                    nc.tensor.matmul(
                                        psy[m][:, ch * 512:(ch + 1) * 512],
                                        wout_sb[:, kt,
                                                m * 128:(m + 1) * 128],
                                        ctxn[kt][:,
                                                 ch * 512:(ch + 1) * 512],
                                        start=(kt == 0), stop=(kt == 3),
                                    )

                    def d_psy(grp):
                        mg = list(range(grp * 2, grp * 2 + 2))
                        if grp % 2 == 0:
                            return mg, {m: sc_tile(f"psy{m}", f"sc{i}")[:]
                                        .rearrange("p a b -> p (a b)")
                                        for i, m in enumerate(mg)}
                        # odd groups borrow the freed psC banks
                        return mg, {m: psC.tile([128, 2, 512], F32,
                                                tag=f"ctx{i}",
                                                name=f"psy{m}")[:]
                                    .rearrange("p a b -> p (a b)")
                                    for i, m in enumerate(mg)}

                    def d_evac(psy, mg):
                        y_sb = tp.tile([128, 2, S], BF16, tag="ysb",
                                       name=f"ysb{mg[0]}")
                        for j, m in enumerate(mg):
                            if m % 2 == 0:
                                nc.scalar.copy(y_sb[:, j, :], psy[m])
                            else:
                                nc.vector.tensor_copy(y_sb[:, j, :], psy[m])
                        nc.sync.dma_start(
                            yT[mg[0] * 128:(mg[0] + 2) * 128, :].rearrange(
                                "(k p) s -> p k s", p=128),
                            y_sb[:])

                    # fill the norm(3) latency: groups 0 and 1 run their
                    # ctxn0-2 slices first, the final slice after the norm
                    mg0, psy0 = d_psy(0)
                    d_mms(psy0, mg0, (0, 1, 2))
                    mg1, psy1 = d_psy(1)
                    d_mms(psy1, mg1, (0, 1, 2))
                    d_mms(psy0, mg0, (3,))
                    d_evac(psy0, mg0)
                    d_mms(psy1, mg1, (3,))
                    d_evac(psy1, mg1)
                    for grp in (2, 3):
                        mg, psy = d_psy(grp)
                        d_mms(psy, mg, range(4))
                        d_evac(psy, mg)

    return nc


def _split_waits(nc, max_waits=1):
    """This walrus build rejects >1 sync-wait command per instruction; hoist
    extra waits onto preceding NoOps on the same engine/queue."""
    for bb in nc.main_func.blocks:
        new_insts = []
        for ins in bb.instructions:
            si = getattr(ins, "sync_info", None)
            if si is not None and si.on_wait and len(si.on_wait) > max_waits:
                waits = list(si.on_wait)
                head, rest = waits[:max_waits], waits[max_waits:]
                while rest:
                    chunk, rest = rest[:max_waits], rest[max_waits:]
                    new_insts.append(mybir.InstNoOp(
                        name=f"waitsplit-{nc.next_id()}", ins=[], outs=[],
                        sync_info=mybir.SyncInfo(on_wait=chunk, on_update=[]),
                        engine=ins.engine))
                ins.sync_info = mybir.SyncInfo(
                    on_wait=head, on_update=list(si.on_update or []))
            new_insts.append(ins)
        bb.instructions = new_insts


def make_core_inputs(x, attention_mask, Wqkv, bqkv, Wout):
    """Host-side shard prep: returns list of 8 in_maps (core = 2*b + g)."""
    Wr = np.ascontiguousarray(Wqkv).reshape(HID, 3, H, D)
    br = np.ascontiguousarray(bqkv).reshape(3, H, D)

    inv = 1.0 / (THETA ** (np.arange(0, D, 2, dtype=np.float64) / D))
    pos = np.arange(S, dtype=np.float64)
    freqs = pos[:, None] * inv[None, :]              # [S, 32]
    emb = np.concatenate([freqs, freqs], axis=1)     # [S, 64]
    cosT = np.cos(emb).T.astype(np.float32)          # [64, S]
    sgn = np.concatenate([-np.ones(32), np.ones(32)])[:, None]
    sinTs = (sgn * np.sin(emb).T).astype(np.float32)
    cos2 = np.concatenate([cosT, cosT], 0)           # [128, S]
    sin2 = np.concatenate([sinTs, sinTs], 0)

    pp = np.arange(128)
    shmap = (pp - pp % 64) + (pp % 64 + 32) % 64
    permT = np.zeros((128, 128), dtype=np.float32)
    permT[shmap, pp] = 1.0

    in_maps = []
    for c in range(NCORES):
        b, g = c // 2, c % 2
        # wqk grouped per head pair: [pair, HID, (q 128 | k 128)]
        wqk_ = np.empty((4, HID, 256), dtype=ml_dtypes.bfloat16)
        for p_ in range(4):
            hs = slice(g * HPC + 2 * p_, g * HPC + 2 * p_ + 2)
            wqk_[p_, :, 0:128] = Wr[:, 0, hs, :].reshape(HID, 128)
            wqk_[p_, :, 128:256] = Wr[:, 1, hs, :].reshape(HID, 128)
        hsg = slice(g * HPC, (g + 1) * HPC)
        wv_ = Wr[:, 2, hsg, :].reshape(HID, 512)
        bqk_ = np.concatenate(
            [br[0, hsg].reshape(512), br[1, hsg].reshape(512)]
        ).reshape(8, 128).T
        bqksh_ = bqk_[shmap]
        mb = (NEGB * (1.0 - attention_mask[b].astype(np.float32))
              ).reshape(ST, 128).T
        in_maps.append({
            "xT": np.ascontiguousarray(x[b].T.astype(ml_dtypes.bfloat16)),
            "wqk": np.ascontiguousarray(wqk_),
            "wv": np.ascontiguousarray(wv_.astype(ml_dtypes.bfloat16)),
            "bqk": np.ascontiguousarray(bqk_.astype(np.float32)),
            "bqksh": np.ascontiguousarray(bqksh_.astype(np.float32)),
            "permT": permT.astype(ml_dtypes.bfloat16),
            "cosT": cos2.astype(ml_dtypes.bfloat16),
            "sinT": sin2.astype(ml_dtypes.bfloat16),
            "mbias": np.ascontiguousarray(mb.astype(np.float32)),
            "wout": np.ascontiguousarray(
                Wout[g * 512:(g + 1) * 512, :].astype(ml_dtypes.bfloat16)),
        })
    return in_maps


_PROGRAM = None


def kernel(x, attention_mask, Wqkv, bqkv, Wout, bout, _trace=False):
    global _PROGRAM
    x = np.asarray(x)
    attention_mask = np.asarray(attention_mask)
    Wqkv = np.asarray(Wqkv)
    bqkv = np.asarray(bqkv)
    Wout = np.asarray(Wout)
    bout = np.asarray(bout)

    if _PROGRAM is None:
        _PROGRAM = build_program()
        _split_waits(_PROGRAM)
    nc = _PROGRAM

    in_maps = make_core_inputs(x, attention_mask, Wqkv, bqkv, Wout)
    res = run_bass_kernel_spmd(
        nc, in_maps, core_ids=list(range(NCORES)), trace=_trace)

    y = np.empty((B, S, HID), dtype=np.float32)
    for b in range(B):
        acc = (res.results[2 * b]["yT"].astype(np.float32)
               + res.results[2 * b + 1]["yT"].astype(np.float32))
        y[b] = acc.T
    # exact host-side bias corrections: v-bias shifts context by a constant
    # (attn rows sum to 1), q/k biases were applied on device.
    bv = bqkv[2 * HID:3 * HID].astype(np.float32)
    y += (bv @ Wout + bout).astype(np.float32)[None, None, :]
    if _trace:
        kernel.last_exec_time_ns = res.exec_time_ns
    return y


# revision 6
# speedup vs baseline: 1.0366x; 1.0010x over previous
"""DeBERTa-RoPE self-attention on 8 Trainium2 cores — v4 (bf16 streaming).

Sharding: data-parallel over batch (4) x tensor-parallel over heads (2 groups
of 8); core = 2*b + g. Host sums the two row-parallel out-proj partials.

All matmul paths run in bf16 (f32 PSUM accumulation); fp8 was measured to
break the 2e-2 gate. The schedule keeps the PE dense end-to-end:

  seg1: qk-proj pairs 0,1 (psum = scores-pool tiles viewed flat)
  w0:   scores+exp pair 0   | qk-proj pairs 2,3 (single rotating bank-pair)
  w1:   scores+exp pair 1   | v-projection (2-bank rolling)
  w2:   scores+exp pair 2   | ctx pair 0 (+softmax-normalize)
  w3:   scores+exp pair 3   | ctx pairs 1,2
  tail: ctx pair 3, out-projection (psum = scores-pool tiles again)

The attention mask folds into the exp bias (masked keys exp to ~0); the
denominator rides as a 65th stationary column of v. exps live in SBUF bf16,
two pair-sized buffers rotating.
"""

import numpy as np
import ml_dtypes

import concourse.bass as bass
import concourse.mybir as mybir
import concourse.tile as tile
from concourse.bass_utils import run_bass_kernel_spmd

H = 16
D = 64
HID = H * D
B = 4
S = 1024
THETA = 10000.0
NCORES = 8
HPC = H // 2          # heads per core
KT = HID // 128       # 8 k-tiles
ST = S // 128         # 8 seq tiles

F32 = mybir.dt.float32
F32R = mybir.dt.float32r
BF16 = mybir.dt.bfloat16
AF = mybir.ActivationFunctionType
ALU = mybir.AluOpType

NEGB = -30.0          # bias for masked keys: exp(s*0.125 - 30) ~ 0


def _r(ap):
    return ap.bitcast(F32R)


def build_program():
    nc = bass.Bass()
    xT = nc.declare_dram_parameter("xT", [HID, S], BF16, isOutput=False)
    wqk = nc.declare_dram_parameter("wqk", [4, HID, 256], BF16, isOutput=False)
    wv = nc.declare_dram_parameter("wv", [HID, 512], BF16, isOutput=False)
    bqk = nc.declare_dram_parameter("bqk", [128, 8], F32, isOutput=False)
    bqksh = nc.declare_dram_parameter("bqksh", [128, 8], F32, isOutput=False)
    cosT = nc.declare_dram_parameter("cosT", [128, S], BF16, isOutput=False)
    sinT = nc.declare_dram_parameter("sinT", [128, S], BF16, isOutput=False)
    mbias = nc.declare_dram_parameter("mbias", [128, ST], F32, isOutput=False)
    wout = nc.declare_dram_parameter("wout", [512, HID], BF16, isOutput=False)
    permT = nc.declare_dram_parameter("permT", [128, 128], BF16, isOutput=False)
    yT = nc.declare_dram_parameter("yT", [HID, S], BF16, isOutput=True)

    with tile.TileContext(nc) as tc, nc.allow_low_precision(
            reason="bf16 recip/cos tables; rel-err budget 2e-2"):
        with (
            tc.tile_pool(name="const", bufs=1) as cpool,
            tc.tile_pool(name="persist", bufs=1) as persist,
        ):
            cos_sb = cpool.tile([128, S], BF16)
            sin_sb = cpool.tile([128, S], BF16)
            mbias_sb = cpool.tile([128, ST], F32)
            bqk_sb = cpool.tile([128, 8], F32)
            bqksh_sb = cpool.tile([128, 8], F32)
            permT_sb = cpool.tile([128, 128], BF16)

            rope = [persist.tile([128, S], BF16, name=f"rope{i}")
                    for i in range(8)]
            # v + denominator-ones column: [t, tt, head, 65]
            vm = persist.tile([128, ST, HPC, 65], BF16)
            ctxn = [persist.tile([128, S], BF16, name=f"ctxn{i}")
                    for i in range(4)]
            xT_sb = persist.tile([128, KT, S], BF16)
            wqk_sb = persist.tile([128, KT, 4, 256], BF16)
            wv_sb = persist.tile([128, KT, 512], BF16)
            wout_sb = persist.tile([128, 4, HID], BF16)

            # ---- input DMAs ----
            nc.scalar.dma_start(cos_sb[:], cosT[:])
            nc.scalar.dma_start(sin_sb[:], sinT[:])
            nc.scalar.dma_start(mbias_sb[:], mbias[:])
            nc.scalar.dma_start(bqk_sb[:], bqk[:])
            nc.scalar.dma_start(bqksh_sb[:], bqksh[:])
            nc.scalar.dma_start(permT_sb[:], permT[:])
            nc.sync.dma_start(
                wqk_sb[:, :, 0, :],
                wqk[0].rearrange("(k p) n -> p k n", p=128))
            for c in range(4):
                nc.sync.dma_start(
                    xT_sb[:, 2 * c:2 * c + 2, :],
                    xT[c * 256:(c + 1) * 256, :].rearrange(
                        "(k p) s -> p k s", p=128))
            nc.sync.dma_start(
                wqk_sb[:, :, 1, :],
                wqk[1].rearrange("(k p) n -> p k n", p=128))
            nc.scalar.dma_start(
                wv_sb[:], wv[:].rearrange("(k p) n -> p k n", p=128))
            for q_ in (2, 3):
                nc.sync.dma_start(
                    wqk_sb[:, :, q_, :],
                    wqk[q_].rearrange("(k p) n -> p k n", p=128))
            nc.sync.dma_start(
                wout_sb[:], wout[:].rearrange("(k p) n -> p k n", p=128))

            # denominator ones-column
            nc.gpsimd.memset(vm[:, :, :, 64:65], 1.0)

            with (
                tc.tile_pool(name="qksb", bufs=1) as qkp,
                tc.tile_pool(name="ropetmp", bufs=1) as rt,
                tc.tile_pool(name="expool", bufs=1) as exp_pool,
                tc.tile_pool(name="tailp", bufs=2) as tp,
                tc.tile_pool(name="small", bufs=1) as small,
                tc.tile_pool(name="rbp", bufs=1) as rbp,
                tc.tile_pool(name="drbounce", bufs=2, space="DRAM") as drb,
                tc.tile_pool(name="psS", bufs=1, space="PSUM") as psS,
            ):
                def sc_tile(name_, tag):
                    return psS.tile([128, 2, 512], F32, tag=tag, name=name_)

                def qk_unit_mms_half(m, ps_qk, half):
                    for kt in range(4 * half, 4 * half + 4):
                        for ch in range(2):
                            nc.tensor.matmul(
                                ps_qk[:, ch * 512:(ch + 1) * 512],
                                wqk_sb[:, kt, m % 4, (m // 4) * 128:
                                       (m // 4) * 128 + 128],
                                xT_sb[:, kt, ch * 512:(ch + 1) * 512],
                                start=(kt == 0), stop=(kt == KT - 1),
                            )

                def qk_unit_mms(m, ps_qk):
                    for kt in range(KT):
                        for ch in range(2):
                            nc.tensor.matmul(
                                ps_qk[:, ch * 512:(ch + 1) * 512],
                                wqk_sb[:, kt, m % 4, (m // 4) * 128:
                                       (m // 4) * 128 + 128],
                                xT_sb[:, kt, ch * 512:(ch + 1) * 512],
                                start=(kt == 0), stop=(kt == KT - 1),
                            )

                def qk_unit_tail(m, ps_qk, psSh):
                    qk_sb = qkp.tile([128, S], BF16, tag="qksb",
                                     name=f"qksb{m}")
                    nc.vector.tensor_copy(qk_sb[:], ps_qk[:])
                    ps_sh = psSh.tile([128, S], F32, tag="sh",
                                      name=f"pssh{m}")
                    for ch in range(2):
                        nc.tensor.matmul(
                            ps_sh[:, ch * 512:(ch + 1) * 512],
                            permT_sb[:],
                            qk_sb[:, ch * 512:(ch + 1) * 512],
                            start=True, stop=True,
                        )
                    for ch in range(2):
                        sl = slice(ch * 512, (ch + 1) * 512)
                        t1 = rt.tile([128, 512], BF16, tag=f"t1{ch}")
                        nc.vector.scalar_tensor_tensor(
                            t1[:], ps_qk[:, sl], bqk_sb[:, m:m + 1],
                            cos_sb[:, sl], op0=ALU.add, op1=ALU.mult)
                        s2 = rt.tile([128, 512], BF16, tag=f"s2{ch}")
                        nc.vector.scalar_tensor_tensor(
                            s2[:], ps_sh[:, sl], bqksh_sb[:, m:m + 1],
                            sin_sb[:, sl], op0=ALU.add, op1=ALU.mult)
                        nc.vector.tensor_add(
                            rope[m][:, sl], t1[:], s2[:])

                def scores_tt(p, tt, ps_sc, ex):
                    qp = rope[p][:]
                    kp = rope[p + 4][:]
                    for ch in range(2):
                        for hh in range(2):
                            base = hh * 64
                            nc.tensor.matmul(
                                ps_sc[ch][:, hh, :],
                                kp[base:base + 64, tt * 128:(tt + 1) * 128],
                                qp[base:base + 64, ch * 512:(ch + 1) * 512],
                                start=True, stop=True,
                                tile_position=(base, 0),
                            )
                        nc.scalar.activation(
                            ex[:, tt, :, ch, :], ps_sc[ch][:],
                            AF.Exp, bias=mbias_sb[:, tt:tt + 1], scale=0.125)

                def new_ex(p):
                    # exps for one pair: [t, tt, hh, ch, s]; 2 rotating bufs
                    return exp_pool.tile([128, ST, 2, 2, 512], BF16,
                                         tag=f"ex{p % 3}", name=f"ex{p}")

                def ctx_alloc(p, psC):
                    return [psC.tile([65, S], F32, tag=f"ctx{hh}",
                                     name=f"ctx{p}_{hh}")
                            for hh in range(2)]

                def ctx_mms_tt(p, ex, ps_cs, tt):
                    for hh in range(2):
                        for ch in range(2):
                            nc.tensor.matmul(
                                ps_cs[hh][:, ch * 512:(ch + 1) * 512],
                                vm[:, tt, 2 * p + hh, :],
                                ex[:, tt, hh, ch, :],
                                start=(tt == 0), stop=(tt == ST - 1),
                            )

                def ctx_norm(p, ps_cs):
                    rc0 = small.tile([1, S], BF16, tag="recip0")
                    rc1 = small.tile([1, S], BF16, tag="recip1")
                    nc.vector.reciprocal(rc0[:], ps_cs[0][64:65, :])
                    nc.vector.reciprocal(rc1[:], ps_cs[1][64:65, :])
                    cu0 = small.tile([64, S], BF16, tag="cun0")
                    cu1 = small.tile([64, S], BF16, tag="cun1")
                    nc.scalar.copy(cu0[:], ps_cs[0][0:64, :])
                    nc.vector.tensor_copy(cu1[:], ps_cs[1][0:64, :])
                    bounce = drb.tile([2, S], BF16)
                    rb = rbp.tile([64, 2, S], BF16, tag="rb")
                    nc.sync.dma_start(bounce[0:1, :], rc0[:])
                    nc.sync.dma_start(
                        rb[:, 0, :], bounce[0:1, :].broadcast_to([64, S]))
                    nc.gpsimd.dma_start(bounce[1:2, :], rc1[:])
                    nc.gpsimd.dma_start(
                        rb[:, 1, :], bounce[1:2, :].broadcast_to([64, S]))
                    nc.vector.tensor_mul(
                        ctxn[p][0:64, :], cu0[:], rb[:, 0, :])
                    nc.vector.tensor_mul(
                        ctxn[p][64:128, :], cu1[:], rb[:, 1, :])

                def ctx_pair(p, ex, psC):
                    ps_cs = ctx_alloc(p, psC)
                    for tt in range(ST):
                        ctx_mms_tt(p, ex, ps_cs, tt)
                    ctx_norm(p, ps_cs)

                # ================= seg 1: qk pairs 0,1 =================
                with (
                    tc.tile_pool(name="psA2", bufs=1, space="PSUM") as psA2,
                    tc.tile_pool(name="psSh", bufs=1, space="PSUM") as psSh,
                ):
                    seg1_ps = {}
                    for i, m in enumerate((4, 0, 5, 1)):
                        # qk psums borrow the scores-pool tags (viewed flat)
                        if i % 2 == 0:
                            ps_q = sc_tile(f"qkps{m}", f"sc{i // 2}")
                        else:
                            ps_q = psA2.tile([128, 2, 512], F32, tag="qk",
                                             name=f"qkps{m}")
                        fl = ps_q[:].rearrange("p a b -> p (a b)")
                        qk_unit_mms(m, fl)
                        qk_unit_tail(m, fl, psSh)

                    # ============ w0: stream pair 0 + qk pairs 2,3 =========
                    ps_sc = (sc_tile("sc0_0", "sc0"), sc_tile("sc1_0", "sc1"))
                    ex0 = new_ex(0)
                    for tt in range(ST):
                        scores_tt(0, tt, ps_sc, ex0)
                        m = (6, 2, 7, 3)[tt // 2]
                        if tt % 2 == 0:
                            psq = psA2.tile([128, 2, 512], F32, tag="qk",
                                            name=f"qkps{m}")
                        qk_unit_mms_half(m, psq[:].rearrange(
                            "p a b -> p (a b)"), tt % 2)
                        if tt % 2 == 1:
                            qk_unit_tail(m, psq[:].rearrange(
                                "p a b -> p (a b)"), psSh)

                # ============ w1: stream pair 1 + v projection =============
                with tc.tile_pool(name="psV", bufs=1, space="PSUM") as psV:
                    ps_sc = (sc_tile("sc0_1", "sc0"), sc_tile("sc1_1", "sc1"))
                    ex1 = new_ex(1)
                    for tt in range(ST):
                        scores_tt(1, tt, ps_sc, ex1)
                        vps = psV.tile([128, 512], F32, tag=f"v{tt % 2}",
                                       name=f"vps{tt}")
                        for kt in range(KT):
                            nc.tensor.matmul(
                                vps[:],
                                xT_sb[:, kt, tt * 128:(tt + 1) * 128],
                                wv_sb[:, kt, :],
                                start=(kt == 0), stop=(kt == KT - 1),
                            )
                        nc.scalar.copy(
                            vm[:, tt, :, 0:64],
                            vps[:].rearrange("p (h d) -> p h d", d=64))

                # ===== w2, w3: stream pairs 2,3 + ctx pairs 0,1,2 ==========
                with tc.tile_pool(name="psC", bufs=1, space="PSUM") as psC:
                    ps_sc = (sc_tile("sc0_2", "sc0"), sc_tile("sc1_2", "sc1"))
                    ex2 = new_ex(2)
                    ps_cs0 = ctx_alloc(0, psC)
                    for tt in range(ST):
                        scores_tt(2, tt, ps_sc, ex2)
                        ctx_mms_tt(0, ex0, ps_cs0, tt)
                    ctx_norm(0, ps_cs0)

                    ps_sc = (sc_tile("sc0_3", "sc0"), sc_tile("sc1_3", "sc1"))
                    ex3 = new_ex(3)
                    ps_cs1 = ctx_alloc(1, psC)
                    for tt in range(ST):
                        scores_tt(3, tt, ps_sc, ex3)
                        ctx_mms_tt(1, ex1, ps_cs1, tt)
                    ctx_norm(1, ps_cs1)
                    ctx_pair(2, ex2, psC)

                    # ================= tail: ctx 3 + out-projection ========
                    ctx_pair(3, ex3, psC)

                    def d_mms(psy, mg, kts):
                        for kt in kts:
                            for m in mg:
                                for ch in range(2):
                                    nc.tensor.matmul(
                                        psy[m][:, ch * 512:(ch + 1) * 512],
                                        wout_sb[:, kt,
                                                m * 128:(m + 1) * 128],
                                        ctxn[kt][:,
                                                 ch * 512:(ch + 1) * 512],
                                        start=(kt == 0), stop=(kt == 3),
                                    )

                    def d_psy(grp):
                        mg = list(range(grp * 2, grp * 2 + 2))
                        if grp % 2 == 0:
                            return mg, {m: sc_tile(f"psy{m}", f"sc{i}")[:]
                                        .rearrange("p a b -> p (a b)")
                                        for i, m in enumerate(mg)}
                        # odd groups borrow the freed psC banks
                        return mg, {m: psC.tile([128, 2, 512], F32,
                                                tag=f"ctx{i}",
                                                name=f"psy{m}")[:]
                                    .rearrange("p a b -> p (a b)")
                                    for i, m in enumerate(mg)}

                    def d_evac(psy, mg, split=False):
                        y_sb = tp.tile([128, 2, S], BF16, tag="ysb",
                                       name=f"ysb{mg[0]}")
                        for j, m in enumerate(mg):
                            if m % 2 == 0:
                                nc.scalar.copy(y_sb[:, j, :], psy[m])
                            else:
                                nc.vector.tensor_copy(y_sb[:, j, :], psy[m])
                            if split:
                                nc.sync.dma_start(
                                    yT[m * 128:(m + 1) * 128, :],
                                    y_sb[:, j, :])
                        if not split:
                            nc.sync.dma_start(
                                yT[mg[0] * 128:(mg[0] + 2) * 128, :].rearrange(
                                    "(k p) s -> p k s", p=128),
                                y_sb[:])

                    # fill the norm(3) latency: groups 0 and 1 run their
                    # ctxn0-2 slices first, the final slice after the norm
                    mg0, psy0 = d_psy(0)
                    d_mms(psy0, mg0, (0, 1, 2))
                    mg1, psy1 = d_psy(1)
                    d_mms(psy1, mg1, (0, 1, 2))
                    d_mms(psy0, mg0, (3,))
                    d_evac(psy0, mg0)
                    d_mms(psy1, mg1, (3,))
                    d_evac(psy1, mg1)
                    for grp in (2, 3):
                        mg, psy = d_psy(grp)
                        d_mms(psy, mg, range(4))
                        d_evac(psy, mg, split=(grp == 3))

    return nc


def _split_waits(nc, max_waits=1):
    """This walrus build rejects >1 sync-wait command per instruction; hoist
    extra waits onto preceding NoOps on the same engine/queue."""
    for bb in nc.main_func.blocks:
        new_insts = []
        for ins in bb.instructions:
            si = getattr(ins, "sync_info", None)
            if si is not None and si.on_wait and len(si.on_wait) > max_waits:
                waits = list(si.on_wait)
                head, rest = waits[:max_waits], waits[max_waits:]
                while rest:
                    chunk, rest = rest[:max_waits], rest[max_waits:]
                    new_insts.append(mybir.InstNoOp(
                        name=f"waitsplit-{nc.next_id()}", ins=[], outs=[],
                        sync_info=mybir.SyncInfo(on_wait=chunk, on_update=[]),
                        engine=ins.engine))
                ins.sync_info = mybir.SyncInfo(
                    on_wait=head, on_update=list(si.on_update or []))
            new_insts.append(ins)
        bb.instructions = new_insts


def make_core_inputs(x, attention_mask, Wqkv, bqkv, Wout):
    """Host-side shard prep: returns list of 8 in_maps (core = 2*b + g)."""
    Wr = np.ascontiguousarray(Wqkv).reshape(HID, 3, H, D)
    br = np.ascontiguousarray(bqkv).reshape(3, H, D)

    inv = 1.0 / (THETA ** (np.arange(0, D, 2, dtype=np.float64) / D))
    pos = np.arange(S, dtype=np.float64)
    freqs = pos[:, None] * inv[None, :]              # [S, 32]
    emb = np.concatenate([freqs, freqs], axis=1)     # [S, 64]
    cosT = np.cos(emb).T.astype(np.float32)          # [64, S]
    sgn = np.concatenate([-np.ones(32), np.ones(32)])[:, None]
    sinTs = (sgn * np.sin(emb).T).astype(np.float32)
    cos2 = np.concatenate([cosT, cosT], 0)           # [128, S]
    sin2 = np.concatenate([sinTs, sinTs], 0)

    pp = np.arange(128)
    shmap = (pp - pp % 64) + (pp % 64 + 32) % 64
    permT = np.zeros((128, 128), dtype=np.float32)
    permT[shmap, pp] = 1.0

    in_maps = []
    for c in range(NCORES):
        b, g = c // 2, c % 2
        # wqk grouped per head pair: [pair, HID, (q 128 | k 128)]
        wqk_ = np.empty((4, HID, 256), dtype=ml_dtypes.bfloat16)
        for p_ in range(4):
            hs = slice(g * HPC + 2 * p_, g * HPC + 2 * p_ + 2)
            wqk_[p_, :, 0:128] = Wr[:, 0, hs, :].reshape(HID, 128)
            wqk_[p_, :, 128:256] = Wr[:, 1, hs, :].reshape(HID, 128)
        hsg = slice(g * HPC, (g + 1) * HPC)
        wv_ = Wr[:, 2, hsg, :].reshape(HID, 512)
        bqk_ = np.concatenate(
            [br[0, hsg].reshape(512), br[1, hsg].reshape(512)]
        ).reshape(8, 128).T
        bqksh_ = bqk_[shmap]
        mb = (NEGB * (1.0 - attention_mask[b].astype(np.float32))
              ).reshape(ST, 128).T
        in_maps.append({
            "xT": np.ascontiguousarray(x[b].T.astype(ml_dtypes.bfloat16)),
            "wqk": np.ascontiguousarray(wqk_),
            "wv": np.ascontiguousarray(wv_.astype(ml_dtypes.bfloat16)),
            "bqk": np.ascontiguousarray(bqk_.astype(np.float32)),
            "bqksh": np.ascontiguousarray(bqksh_.astype(np.float32)),
            "permT": permT.astype(ml_dtypes.bfloat16),
            "cosT": cos2.astype(ml_dtypes.bfloat16),
            "sinT": sin2.astype(ml_dtypes.bfloat16),
            "mbias": np.ascontiguousarray(mb.astype(np.float32)),
            "wout": np.ascontiguousarray(
                Wout[g * 512:(g + 1) * 512, :].astype(ml_dtypes.bfloat16)),
        })
    return in_maps


_PROGRAM = None


def kernel(x, attention_mask, Wqkv, bqkv, Wout, bout, _trace=False):
    global _PROGRAM
    x = np.asarray(x)
    attention_mask = np.asarray(attention_mask)
    Wqkv = np.asarray(Wqkv)
    bqkv = np.asarray(bqkv)
    Wout = np.asarray(Wout)
    bout = np.asarray(bout)

    if _PROGRAM is None:
        _PROGRAM = build_program()
        _split_waits(_PROGRAM)
    nc = _PROGRAM

    in_maps = make_core_inputs(x, attention_mask, Wqkv, bqkv, Wout)
    res = run_bass_kernel_spmd(
        nc, in_maps, core_ids=list(range(NCORES)), trace=_trace)

    y = np.empty((B, S, HID), dtype=np.float32)
    for b in range(B):
        acc = (res.results[2 * b]["yT"].astype(np.float32)
               + res.results[2 * b + 1]["yT"].astype(np.float32))
        y[b] = acc.T
    # exact host-side bias corrections: v-bias shifts context by a constant
    # (attn rows sum to 1), q/k biases were applied on device.
    bv = bqkv[2 * HID:3 * HID].astype(np.float32)
    y += (bv @ Wout + bout).astype(np.float32)[None, None, :]
    if _trace:
        kernel.last_exec_time_ns = res.exec_time_ns
    return y


# revision 7
# speedup vs baseline: 1.0430x; 1.0062x over previous
"""DeBERTa-RoPE self-attention on 8 Trainium2 cores — v4 (bf16 streaming).

Sharding: data-parallel over batch (4) x tensor-parallel over heads (2 groups
of 8); core = 2*b + g. Host sums the two row-parallel out-proj partials.

All matmul paths run in bf16 (f32 PSUM accumulation); fp8 was measured to
break the 2e-2 gate. The schedule keeps the PE dense end-to-end:

  seg1: qk-proj pairs 0,1 (psum = scores-pool tiles viewed flat)
  w0:   scores+exp pair 0   | qk-proj pairs 2,3 (single rotating bank-pair)
  w1:   scores+exp pair 1   | v-projection (2-bank rolling)
  w2:   scores+exp pair 2   | ctx pair 0 (+softmax-normalize)
  w3:   scores+exp pair 3   | ctx pairs 1,2
  tail: ctx pair 3, out-projection (psum = scores-pool tiles again)

The attention mask folds into the exp bias (masked keys exp to ~0); the
denominator rides as a 65th stationary column of v. exps live in SBUF bf16,
two pair-sized buffers rotating.
"""

import numpy as np
import ml_dtypes

import concourse.bass as bass
import concourse.mybir as mybir
import concourse.tile as tile
from concourse.bass_utils import run_bass_kernel_spmd

H = 16
D = 64
HID = H * D
B = 4
S = 1024
THETA = 10000.0
NCORES = 8
HPC = H // 2          # heads per core
KT = HID // 128       # 8 k-tiles
ST = S // 128         # 8 seq tiles

F32 = mybir.dt.float32
F32R = mybir.dt.float32r
BF16 = mybir.dt.bfloat16
AF = mybir.ActivationFunctionType
ALU = mybir.AluOpType

NEGB = -30.0          # bias for masked keys: exp(s*0.125 - 30) ~ 0


def _r(ap):
    return ap.bitcast(F32R)


def build_program():
    nc = bass.Bass()
    xT = nc.declare_dram_parameter("xT", [HID, S], BF16, isOutput=False)
    wqk = nc.declare_dram_parameter("wqk", [4, HID, 256], BF16, isOutput=False)
    wv = nc.declare_dram_parameter("wv", [HID, 512], BF16, isOutput=False)
    bqk = nc.declare_dram_parameter("bqk", [128, 8], F32, isOutput=False)
    bqksh = nc.declare_dram_parameter("bqksh", [128, 8], F32, isOutput=False)
    cosT = nc.declare_dram_parameter("cosT", [128, S], BF16, isOutput=False)
    sinT = nc.declare_dram_parameter("sinT", [128, S], BF16, isOutput=False)
    mbias = nc.declare_dram_parameter("mbias", [128, ST], F32, isOutput=False)
    wout = nc.declare_dram_parameter("wout", [512, HID], BF16, isOutput=False)
    permT = nc.declare_dram_parameter("permT", [128, 128], BF16, isOutput=False)
    yT = nc.declare_dram_parameter("yT", [HID, S], BF16, isOutput=True)

    with tile.TileContext(nc) as tc, nc.allow_low_precision(
            reason="bf16 recip/cos tables; rel-err budget 2e-2"):
        with (
            tc.tile_pool(name="const", bufs=1) as cpool,
            tc.tile_pool(name="persist", bufs=1) as persist,
        ):
            cos_sb = cpool.tile([128, S], BF16)
            sin_sb = cpool.tile([128, S], BF16)
            mbias_sb = cpool.tile([128, ST], F32)
            bqk_sb = cpool.tile([128, 8], F32)
            bqksh_sb = cpool.tile([128, 8], F32)
            permT_sb = cpool.tile([128, 128], BF16)

            rope = [persist.tile([128, S], BF16, name=f"rope{i}")
                    for i in range(8)]
            # v + denominator-ones column: [t, tt, head, 65]
            vm = persist.tile([128, ST, HPC, 65], BF16)
            ctxn = [persist.tile([128, S], BF16, name=f"ctxn{i}")
                    for i in range(4)]
            xT_sb = persist.tile([128, KT, S], BF16)
            wqk_sb = persist.tile([128, KT, 4, 256], BF16)
            wv_sb = persist.tile([128, KT, 512], BF16)
            wout_sb = persist.tile([128, 4, HID], BF16)

            # ---- input DMAs ----
            nc.scalar.dma_start(cos_sb[:], cosT[:])
            nc.scalar.dma_start(sin_sb[:], sinT[:])
            nc.scalar.dma_start(mbias_sb[:], mbias[:])
            nc.scalar.dma_start(bqk_sb[:], bqk[:])
            nc.scalar.dma_start(bqksh_sb[:], bqksh[:])
            nc.scalar.dma_start(permT_sb[:], permT[:])
            nc.sync.dma_start(
                wqk_sb[:, :, 0, :],
                wqk[0].rearrange("(k p) n -> p k n", p=128))
            for c in range(4):
                nc.sync.dma_start(
                    xT_sb[:, 2 * c:2 * c + 2, :],
                    xT[c * 256:(c + 1) * 256, :].rearrange(
                        "(k p) s -> p k s", p=128))
            nc.sync.dma_start(
                wqk_sb[:, :, 1, :],
                wqk[1].rearrange("(k p) n -> p k n", p=128))
            nc.scalar.dma_start(
                wv_sb[:], wv[:].rearrange("(k p) n -> p k n", p=128))
            for q_ in (2, 3):
                nc.sync.dma_start(
                    wqk_sb[:, :, q_, :],
                    wqk[q_].rearrange("(k p) n -> p k n", p=128))
            nc.sync.dma_start(
                wout_sb[:], wout[:].rearrange("(k p) n -> p k n", p=128))

            # denominator ones-column
            nc.gpsimd.memset(vm[:, :, :, 64:65], 1.0)

            with (
                tc.tile_pool(name="qksb", bufs=1) as qkp,
                tc.tile_pool(name="ropetmp", bufs=1) as rt,
                tc.tile_pool(name="expool", bufs=1) as exp_pool,
                tc.tile_pool(name="tailp", bufs=2) as tp,
                tc.tile_pool(name="small", bufs=1) as small,
                tc.tile_pool(name="rbp", bufs=1) as rbp,
                tc.tile_pool(name="drbounce", bufs=2, space="DRAM") as drb,
                tc.tile_pool(name="psS", bufs=1, space="PSUM") as psS,
            ):
                def sc_tile(name_, tag):
                    return psS.tile([128, 2, 512], F32, tag=tag, name=name_)

                def qk_unit_mms_half(m, ps_qk, half):
                    for kt in range(4 * half, 4 * half + 4):
                        for ch in range(2):
                            nc.tensor.matmul(
                                ps_qk[:, ch * 512:(ch + 1) * 512],
                                wqk_sb[:, kt, m % 4, (m // 4) * 128:
                                       (m // 4) * 128 + 128],
                                xT_sb[:, kt, ch * 512:(ch + 1) * 512],
                                start=(kt == 0), stop=(kt == KT - 1),
                            )

                def qk_unit_mms(m, ps_qk):
                    for kt in range(KT):
                        for ch in range(2):
                            nc.tensor.matmul(
                                ps_qk[:, ch * 512:(ch + 1) * 512],
                                wqk_sb[:, kt, m % 4, (m // 4) * 128:
                                       (m // 4) * 128 + 128],
                                xT_sb[:, kt, ch * 512:(ch + 1) * 512],
                                start=(kt == 0), stop=(kt == KT - 1),
                            )

                def qk_unit_tail(m, ps_qk, psSh):
                    qk_sb = qkp.tile([128, S], BF16, tag="qksb",
                                     name=f"qksb{m}")
                    nc.vector.tensor_copy(qk_sb[:], ps_qk[:])
                    ps_sh = psSh.tile([128, S], F32, tag="sh",
                                      name=f"pssh{m}")
                    for ch in range(2):
                        nc.tensor.matmul(
                            ps_sh[:, ch * 512:(ch + 1) * 512],
                            permT_sb[:],
                            qk_sb[:, ch * 512:(ch + 1) * 512],
                            start=True, stop=True,
                        )
                    for ch in range(2):
                        sl = slice(ch * 512, (ch + 1) * 512)
                        t1 = rt.tile([128, 512], BF16, tag=f"t1{ch}")
                        nc.vector.scalar_tensor_tensor(
                            t1[:], ps_qk[:, sl], bqk_sb[:, m:m + 1],
                            cos_sb[:, sl], op0=ALU.add, op1=ALU.mult)
                        s2 = rt.tile([128, 512], BF16, tag=f"s2{ch}")
                        nc.vector.scalar_tensor_tensor(
                            s2[:], ps_sh[:, sl], bqksh_sb[:, m:m + 1],
                            sin_sb[:, sl], op0=ALU.add, op1=ALU.mult)
                        nc.vector.tensor_add(
                            rope[m][:, sl], t1[:], s2[:])

                def scores_tt(p, tt, ps_sc, ex):
                    qp = rope[p][:]
                    kp = rope[p + 4][:]
                    for ch in range(2):
                        for hh in range(2):
                            base = hh * 64
                            nc.tensor.matmul(
                                ps_sc[ch][:, hh, :],
                                kp[base:base + 64, tt * 128:(tt + 1) * 128],
                                qp[base:base + 64, ch * 512:(ch + 1) * 512],
                                start=True, stop=True,
                                tile_position=(base, 0),
                            )
                        nc.scalar.activation(
                            ex[:, tt, :, ch, :], ps_sc[ch][:],
                            AF.Exp, bias=mbias_sb[:, tt:tt + 1], scale=0.125)

                def new_ex(p):
                    # exps for one pair: [t, tt, hh, ch, s]; 2 rotating bufs
                    return exp_pool.tile([128, ST, 2, 2, 512], BF16,
                                         tag=f"ex{p % 3}", name=f"ex{p}")

                def ctx_alloc(p, psC):
                    return [psC.tile([65, S], F32, tag=f"ctx{hh}",
                                     name=f"ctx{p}_{hh}")
                            for hh in range(2)]

                def ctx_mms_tt(p, ex, ps_cs, tt):
                    for hh in range(2):
                        for ch in range(2):
                            nc.tensor.matmul(
                                ps_cs[hh][:, ch * 512:(ch + 1) * 512],
                                vm[:, tt, 2 * p + hh, :],
                                ex[:, tt, hh, ch, :],
                                start=(tt == 0), stop=(tt == ST - 1),
                            )

                def ctx_norm(p, ps_cs):
                    rc0 = small.tile([1, S], BF16, tag="recip0")
                    rc1 = small.tile([1, S], BF16, tag="recip1")
                    nc.vector.reciprocal(rc0[:], ps_cs[0][64:65, :])
                    nc.vector.reciprocal(rc1[:], ps_cs[1][64:65, :])
                    cu0 = small.tile([64, S], BF16, tag="cun0")
                    cu1 = small.tile([64, S], BF16, tag="cun1")
                    nc.scalar.copy(cu0[:], ps_cs[0][0:64, :])
                    nc.vector.tensor_copy(cu1[:], ps_cs[1][0:64, :])
                    bounce = drb.tile([2, S], BF16)
                    rb = rbp.tile([64, 2, S], BF16, tag="rb")
                    nc.sync.dma_start(bounce[0:1, :], rc0[:])
                    nc.sync.dma_start(
                        rb[:, 0, :], bounce[0:1, :].broadcast_to([64, S]))
                    nc.gpsimd.dma_start(bounce[1:2, :], rc1[:])
                    nc.gpsimd.dma_start(
                        rb[:, 1, :], bounce[1:2, :].broadcast_to([64, S]))
                    nc.vector.tensor_mul(
                        ctxn[p][0:64, :], cu0[:], rb[:, 0, :])
                    nc.vector.tensor_mul(
                        ctxn[p][64:128, :], cu1[:], rb[:, 1, :])

                def ctx_pair(p, ex, psC):
                    ps_cs = ctx_alloc(p, psC)
                    for tt in range(ST):
                        ctx_mms_tt(p, ex, ps_cs, tt)
                    ctx_norm(p, ps_cs)

                # ================= seg 1: qk pairs 0,1 =================
                with (
                    tc.tile_pool(name="psA2", bufs=1, space="PSUM") as psA2,
                    tc.tile_pool(name="psSh", bufs=1, space="PSUM") as psSh,
                ):
                    seg1_ps = {}
                    for i, m in enumerate((4, 0, 5, 1)):
                        # qk psums borrow the scores-pool tags (viewed flat)
                        if i % 2 == 0:
                            ps_q = sc_tile(f"qkps{m}", f"sc{i // 2}")
                        else:
                            ps_q = psA2.tile([128, 2, 512], F32, tag="qk",
                                             name=f"qkps{m}")
                        fl = ps_q[:].rearrange("p a b -> p (a b)")
                        qk_unit_mms(m, fl)
                        qk_unit_tail(m, fl, psSh)

                    # ============ w0: stream pair 0 + qk pairs 2,3 =========
                    ps_sc = (sc_tile("sc0_0", "sc0"), sc_tile("sc1_0", "sc1"))
                    ex0 = new_ex(0)
                    for tt in range(ST):
                        scores_tt(0, tt, ps_sc, ex0)
                        if tt == 0:
                            continue
                        m = (6, 2, 7, 3)[(tt - 1) // 2]
                        if (tt - 1) % 2 == 0:
                            psq = psA2.tile([128, 2, 512], F32, tag="qk",
                                            name=f"qkps{m}")
                        qk_unit_mms_half(m, psq[:].rearrange(
                            "p a b -> p (a b)"), (tt - 1) % 2)
                        if (tt - 1) % 2 == 1:
                            qk_unit_tail(m, psq[:].rearrange(
                                "p a b -> p (a b)"), psSh)
                    qk_unit_mms_half(3, psq[:].rearrange(
                        "p a b -> p (a b)"), 1)
                    qk_unit_tail(3, psq[:].rearrange("p a b -> p (a b)"),
                                 psSh)

                # ============ w1: stream pair 1 + v projection =============
                with tc.tile_pool(name="psV", bufs=1, space="PSUM") as psV:
                    ps_sc = (sc_tile("sc0_1", "sc0"), sc_tile("sc1_1", "sc1"))
                    ex1 = new_ex(1)
                    for tt in range(ST):
                        scores_tt(1, tt, ps_sc, ex1)
                        vps = psV.tile([128, 512], F32, tag=f"v{tt % 2}",
                                       name=f"vps{tt}")
                        for kt in range(KT):
                            nc.tensor.matmul(
                                vps[:],
                                xT_sb[:, kt, tt * 128:(tt + 1) * 128],
                                wv_sb[:, kt, :],
                                start=(kt == 0), stop=(kt == KT - 1),
                            )
                        nc.scalar.copy(
                            vm[:, tt, :, 0:64],
                            vps[:].rearrange("p (h d) -> p h d", d=64))

                # ===== w2, w3: stream pairs 2,3 + ctx pairs 0,1,2 ==========
                with tc.tile_pool(name="psC", bufs=1, space="PSUM") as psC:
                    ps_sc = (sc_tile("sc0_2", "sc0"), sc_tile("sc1_2", "sc1"))
                    ex2 = new_ex(2)
                    ps_cs0 = ctx_alloc(0, psC)
                    for tt in range(ST):
                        scores_tt(2, tt, ps_sc, ex2)
                        ctx_mms_tt(0, ex0, ps_cs0, tt)
                    ctx_norm(0, ps_cs0)

                    ps_sc = (sc_tile("sc0_3", "sc0"), sc_tile("sc1_3", "sc1"))
                    ex3 = new_ex(3)
                    ps_cs1 = ctx_alloc(1, psC)
                    for tt in range(ST):
                        scores_tt(3, tt, ps_sc, ex3)
                        ctx_mms_tt(1, ex1, ps_cs1, tt)
                    ctx_norm(1, ps_cs1)
                    ctx_pair(2, ex2, psC)

                    # ================= tail: ctx 3 + out-projection ========
                    ctx_pair(3, ex3, psC)

                    def d_mms(psy, mg, kts):
                        for kt in kts:
                            for m in mg:
                                for ch in range(2):
                                    nc.tensor.matmul(
                                        psy[m][:, ch * 512:(ch + 1) * 512],
                                        wout_sb[:, kt,
                                                m * 128:(m + 1) * 128],
                                        ctxn[kt][:,
                                                 ch * 512:(ch + 1) * 512],
                                        start=(kt == 0), stop=(kt == 3),
                                    )

                    def d_psy(grp):
                        mg = list(range(grp * 2, grp * 2 + 2))
                        if grp % 2 == 0:
                            return mg, {m: sc_tile(f"psy{m}", f"sc{i}")[:]
                                        .rearrange("p a b -> p (a b)")
                                        for i, m in enumerate(mg)}
                        # odd groups borrow the freed psC banks
                        return mg, {m: psC.tile([128, 2, 512], F32,
                                                tag=f"ctx{i}",
                                                name=f"psy{m}")[:]
                                    .rearrange("p a b -> p (a b)")
                                    for i, m in enumerate(mg)}

                    def d_evac(psy, mg, split=False):
                        y_sb = tp.tile([128, 2, S], BF16, tag="ysb",
                                       name=f"ysb{mg[0]}")
                        for j, m in enumerate(mg):
                            if m % 2 == 0:
                                nc.scalar.copy(y_sb[:, j, :], psy[m])
                            else:
                                nc.vector.tensor_copy(y_sb[:, j, :], psy[m])
                            if split:
                                nc.sync.dma_start(
                                    yT[m * 128:(m + 1) * 128, :],
                                    y_sb[:, j, :])
                        if not split:
                            nc.sync.dma_start(
                                yT[mg[0] * 128:(mg[0] + 2) * 128, :].rearrange(
                                    "(k p) s -> p k s", p=128),
                                y_sb[:])

                    # fill the norm(3) latency: groups 0 and 1 run their
                    # ctxn0-2 slices first, the final slice after the norm
                    mg0, psy0 = d_psy(0)
                    d_mms(psy0, mg0, (0, 1, 2))
                    mg1, psy1 = d_psy(1)
                    d_mms(psy1, mg1, (0, 1, 2))
                    d_mms(psy0, mg0, (3,))
                    d_evac(psy0, mg0)
                    d_mms(psy1, mg1, (3,))
                    d_evac(psy1, mg1)
                    for grp in (2, 3):
                        mg, psy = d_psy(grp)
                        d_mms(psy, mg, range(4))
                        d_evac(psy, mg, split=(grp == 3))

    return nc


def _split_waits(nc, max_waits=1):
    """This walrus build rejects >1 sync-wait command per instruction; hoist
    extra waits onto preceding NoOps on the same engine/queue."""
    for bb in nc.main_func.blocks:
        new_insts = []
        for ins in bb.instructions:
            si = getattr(ins, "sync_info", None)
            if si is not None and si.on_wait and len(si.on_wait) > max_waits:
                waits = list(si.on_wait)
                head, rest = waits[:max_waits], waits[max_waits:]
                while rest:
                    chunk, rest = rest[:max_waits], rest[max_waits:]
                    new_insts.append(mybir.InstNoOp(
                        name=f"waitsplit-{nc.next_id()}", ins=[], outs=[],
                        sync_info=mybir.SyncInfo(on_wait=chunk, on_update=[]),
                        engine=ins.engine))
                ins.sync_info = mybir.SyncInfo(
                    on_wait=head, on_update=list(si.on_update or []))
            new_insts.append(ins)
        bb.instructions = new_insts


def make_core_inputs(x, attention_mask, Wqkv, bqkv, Wout):
    """Host-side shard prep: returns list of 8 in_maps (core = 2*b + g)."""
    Wr = np.ascontiguousarray(Wqkv).reshape(HID, 3, H, D)
    br = np.ascontiguousarray(bqkv).reshape(3, H, D)

    inv = 1.0 / (THETA ** (np.arange(0, D, 2, dtype=np.float64) / D))
    pos = np.arange(S, dtype=np.float64)
    freqs = pos[:, None] * inv[None, :]              # [S, 32]
    emb = np.concatenate([freqs, freqs], axis=1)     # [S, 64]
    cosT = np.cos(emb).T.astype(np.float32)          # [64, S]
    sgn = np.concatenate([-np.ones(32), np.ones(32)])[:, None]
    sinTs = (sgn * np.sin(emb).T).astype(np.float32)
    cos2 = np.concatenate([cosT, cosT], 0)           # [128, S]
    sin2 = np.concatenate([sinTs, sinTs], 0)

    pp = np.arange(128)
    shmap = (pp - pp % 64) + (pp % 64 + 32) % 64
    permT = np.zeros((128, 128), dtype=np.float32)
    permT[shmap, pp] = 1.0

    in_maps = []
    for c in range(NCORES):
        b, g = c // 2, c % 2
        # wqk grouped per head pair: [pair, HID, (q 128 | k 128)]
        wqk_ = np.empty((4, HID, 256), dtype=ml_dtypes.bfloat16)
        for p_ in range(4):
            hs = slice(g * HPC + 2 * p_, g * HPC + 2 * p_ + 2)
            wqk_[p_, :, 0:128] = Wr[:, 0, hs, :].reshape(HID, 128)
            wqk_[p_, :, 128:256] = Wr[:, 1, hs, :].reshape(HID, 128)
        hsg = slice(g * HPC, (g + 1) * HPC)
        wv_ = Wr[:, 2, hsg, :].reshape(HID, 512)
        bqk_ = np.concatenate(
            [br[0, hsg].reshape(512), br[1, hsg].reshape(512)]
        ).reshape(8, 128).T
        bqksh_ = bqk_[shmap]
        mb = (NEGB * (1.0 - attention_mask[b].astype(np.float32))
              ).reshape(ST, 128).T
        in_maps.append({
            "xT": np.ascontiguousarray(x[b].T.astype(ml_dtypes.bfloat16)),
            "wqk": np.ascontiguousarray(wqk_),
            "wv": np.ascontiguousarray(wv_.astype(ml_dtypes.bfloat16)),
            "bqk": np.ascontiguousarray(bqk_.astype(np.float32)),
            "bqksh": np.ascontiguousarray(bqksh_.astype(np.float32)),
            "permT": permT.astype(ml_dtypes.bfloat16),
            "cosT": cos2.astype(ml_dtypes.bfloat16),
            "sinT": sin2.astype(ml_dtypes.bfloat16),
            "mbias": np.ascontiguousarray(mb.astype(np.float32)),
            "wout": np.ascontiguousarray(
                Wout[g * 512:(g + 1) * 512, :].astype(ml_dtypes.bfloat16)),
        })
    return in_maps


_PROGRAM = None


def kernel(x, attention_mask, Wqkv, bqkv, Wout, bout, _trace=False):
    global _PROGRAM
    x = np.asarray(x)
    attention_mask = np.asarray(attention_mask)
    Wqkv = np.asarray(Wqkv)
    bqkv = np.asarray(bqkv)
    Wout = np.asarray(Wout)
    bout = np.asarray(bout)

    if _PROGRAM is None:
        _PROGRAM = build_program()
        _split_waits(_PROGRAM)
    nc = _PROGRAM

    in_maps = make_core_inputs(x, attention_mask, Wqkv, bqkv, Wout)
    res = run_bass_kernel_spmd(
        nc, in_maps, core_ids=list(range(NCORES)), trace=_trace)

    y = np.empty((B, S, HID), dtype=np.float32)
    for b in range(B):
        acc = (res.results[2 * b]["yT"].astype(np.float32)
               + res.results[2 * b + 1]["yT"].astype(np.float32))
        y[b] = acc.T
    # exact host-side bias corrections: v-bias shifts context by a constant
    # (attn rows sum to 1), q/k biases were applied on device.
    bv = bqkv[2 * HID:3 * HID].astype(np.float32)
    y += (bv @ Wout + bout).astype(np.float32)[None, None, :]
    if _trace:
        kernel.last_exec_time_ns = res.exec_time_ns
    return y
